# revision 32
# baseline (speedup 1.0000x reference)
"""Trainium2 Bass kernel for BailingMoeV2.5 linear attention (simple GLA).

Sharding: tensor-parallel over heads. 8 cores x 2 heads each.
  - qkv + gate projections: per-core output-column shards, transposed-hidden
    (precomputed on host, bf16) as the stationary matmul operand.
  - q/k RMSNorm + partial RoPE fused into the projection epilogue.
  - chunked simple-GLA scan (chunk=128), sequential over 64 chunks,
    embarrassingly parallel over heads; fp32 state, bf16 matmuls.
  - GroupRMSNorm group == the core's own 2 heads (local); sigmoid gate.
  - AllGather of bf16 attention output (4.2MB/core), then each core computes
    a 256-column slice of the dense projection (g_norm_w folded into w_dense).
All matmuls bf16 with fp32 PSUM accumulation.
"""
import math
import numpy as np
import ml_dtypes

import concourse.bass as bass
import concourse.bacc as bacc
import concourse.mybir as mybir
import concourse.tile as tile
import concourse.bass_utils as bass_utils

BF16_NP = ml_dtypes.bfloat16
DT = mybir.dt
AF = mybir.ActivationFunctionType
ALU = mybir.AluOpType

T, H, D, HID = 8192, 16, 128, 2048
RD = 64
THETA = 10000.0
EPS = 1e-6
LAYER_IDX, N_LAYERS = 12, 32
CHUNK = 128
NCH = T // CHUNK            # 64 chunks
N_CORES = 8
HPC = H // N_CORES          # 2 heads per core
JS = HID // N_CORES         # 256 output cols per core in dense
TB = 4                      # chunks per t-block (512 t per block)
NTB = NCH // TB             # 16 t-blocks
KT = HID // 128             # 16 k-tiles


def _slopes():
    start = 2.0 ** (-(2.0 ** (-(math.log2(H) - 3))))
    base = np.array([start * start ** i for i in range(H)], dtype=np.float64)
    return base * (-(1.0 - LAYER_IDX / (N_LAYERS - 1) + 1e-5))


def _build_nc(apply_norm_w: bool):
    nc = bacc.Bacc("TRN2", target_bir_lowering=False, debug=False,
                   enable_asserts=False, num_devices=N_CORES)

    f32, bf16 = DT.float32, DT.bfloat16

    # ---- I/O ----
    hT_b = nc.dram_tensor("hT_b", [HID, T], bf16, kind="ExternalInput")
    w_pack = nc.dram_tensor("w_pack", [HID, 4 * HPC * D], bf16, kind="ExternalInput")
    w_dense_sc = nc.dram_tensor("w_dense_sc", [HID, HID], bf16, kind="ExternalInput")
    cos4 = nc.dram_tensor("cos4", [T, 128], f32, kind="ExternalInput")
    sin4 = nc.dram_tensor("sin4", [T, 128], f32, kind="ExternalInput")
    adect_in = nc.dram_tensor("adect", [HPC * CHUNK, CHUNK], f32, kind="ExternalInput")
    qdecb_in = nc.dram_tensor("qdecb", [HPC * D, CHUNK], f32, kind="ExternalInput")
    kdec_in = nc.dram_tensor("kdec", [CHUNK, HPC], f32, kind="ExternalInput")
    sdec_in = nc.dram_tensor("sdec", [CHUNK, HPC], f32, kind="ExternalInput")
    state_in = nc.dram_tensor("state_in", [HPC * D, D], f32, kind="ExternalInput")
    ident_in = nc.dram_tensor("ident", [128, 128], bf16, kind="ExternalInput")
    if apply_norm_w:
        normw_in = nc.dram_tensor("normw", [CHUNK, 512], f32, kind="ExternalInput")

    TS = T // N_CORES  # 1024: t-slice per core after all-to-all
    out_slice = nc.dram_tensor("out_slice", [TS, HID], f32, kind="ExternalOutput")
    state_out = nc.dram_tensor("state_out", [HPC * D, D], f32, kind="ExternalOutput")

    # ---- internal DRAM ----
    # three t-segments, one all-to-all each (separate tensors so collective
    # deps don't serialize the segments): chunks [0,32), [32,48), [48,64)
    SEG_CH = [(0, 32), (32, 48), (48, 64)]     # chunk ranges
    SEG_ROWS = [(hi - lo) * CHUNK for lo, hi in SEG_CH]
    SEG_SHARD = [r // N_CORES for r in SEG_ROWS]  # per-rank rows: 512/256/256
    attn_seg = [nc.dram_tensor(f"attn_loc{i}", [SEG_ROWS[i], HPC * D], bf16)
                for i in range(3)]
    att_x = [nc.dram_tensor(f"att_x{i}", [SEG_ROWS[i], HPC * D], bf16)
             for i in range(3)]

    with tile.TileContext(nc) as tc:
        with (
            tc.tile_pool(name="const", bufs=1) as cpool,
            tc.tile_pool(name="densew", bufs=1) as dwp,
            tc.tile_pool(name="densework", bufs=1) as dwork,
        ):
            S_sb = cpool.tile([128, HPC * D], f32, tag="S_sb")
            S_b = cpool.tile([128, HPC * D], bf16, tag="S_b")
            adect_sb = cpool.tile([128, HPC * CHUNK], f32, tag="adect")
            qdecb_sb = cpool.tile([128, HPC * CHUNK], f32, tag="qdecb")
            kdec_sb = cpool.tile([128, HPC], f32, tag="kdec")
            sdec_sb = cpool.tile([128, HPC], f32, tag="sdec")
            if apply_norm_w:
                normw_sb = cpool.tile([128, 512], f32, tag="normw")
                nc.sync.dma_start(normw_sb[:], normw_in[:, :])
            ident_sb = cpool.tile([128, 128], bf16, tag="ident")
            nc.sync.dma_start(ident_sb[:], ident_in[:, :])
            eps_sb = cpool.tile([128, 1], f32, tag="eps")
            zero_sb = cpool.tile([128, 1], f32, tag="zero")
            nc.vector.memset(eps_sb[:], EPS)
            nc.vector.memset(zero_sb[:], 0.0)

            for h in range(HPC):
                nc.sync.dma_start(S_sb[:, h * D:(h + 1) * D],
                                  state_in[h * D:(h + 1) * D, :])
                nc.sync.dma_start(adect_sb[:, h * CHUNK:(h + 1) * CHUNK],
                                  adect_in[h * CHUNK:(h + 1) * CHUNK, :])
                nc.sync.dma_start(qdecb_sb[:, h * CHUNK:(h + 1) * CHUNK],
                                  qdecb_in[h * D:(h + 1) * D, :])
            nc.sync.dma_start(kdec_sb[:], kdec_in[:, :])
            nc.sync.dma_start(sdec_sb[:], sdec_in[:, :])
            nc.vector.tensor_copy(S_b[:], S_sb[:])

            # dense weights: tile at top-level scope, loads emitted later
            wd_sb = dwp.tile([128, KT * HID], bf16, tag="wd_sb")

            aT_anchor = [None, None]

            def emit_aT(seg, anchor=None):
                rows = SEG_SHARD[seg]
                aT = dwork.tile([128, KT * rows], bf16, tag=f"aT{seg}", bufs=1,
                                name=f"aT_{seg}")
                for i in range(KT):
                    # full-attn col block i lives at att_x[seg] rows
                    # (i//2)*rows + t_rel, cols (i%2)*128
                    r0 = (i // HPC) * rows
                    c0 = (i % HPC) * 128
                    tp = nc.sync.dma_start_transpose(
                        aT[:, i * rows:(i + 1) * rows],
                        att_x[seg][r0:r0 + rows, c0:c0 + 128])
                    if anchor is not None:
                        # keep the scheduler from hoisting these ahead of the
                        # collective (its cost model underestimates CC time,
                        # and a head-of-line blocked transpose starves the
                        # whole sync queue)
                        tile.add_dep_helper(tp.ins, anchor.ins,
                                            reason="aT after late-phase anchor")
                return aT

            def emit_dense(seg, aT, pdp):
                rows = SEG_SHARD[seg]
                rbase = sum(SEG_SHARD[:seg])
                for tt in range(rows // 128):
                    for jb in range(4):
                        psD = pdp.tile([128, 512], f32, tag="psD",
                                       name=f"psD_{seg}_{tt}_{jb}")
                        for i in range(KT):
                            lhs = aT[:, i * rows + tt * 128: i * rows + (tt + 1) * 128]
                            nc.tensor.matmul(
                                psD[:], lhs,
                                wd_sb[:, i * HID + jb * 512: i * HID + (jb + 1) * 512],
                                start=(i == 0), stop=(i == KT - 1))
                        oc = dwork.tile([128, 512], f32, tag="oc", bufs=2,
                                        name=f"oc_{seg}_{tt}_{jb}")
                        nc.vector.tensor_copy(oc[:], psD[:])
                        r = rbase + tt * 128
                        nc.sync.dma_start(
                            out_slice[r:r + 128, jb * 512:(jb + 1) * 512], oc[:])

            with (
                tc.tile_pool(name="big", bufs=1) as bigp,
                tc.tile_pool(name="ring", bufs=3) as ringp,
                tc.tile_pool(name="work", bufs=2) as workp,
                tc.tile_pool(name="psA", bufs=2, space="PSUM") as pap,
                tc.tile_pool(name="psB", bufs=2, space="PSUM") as pbp,
                tc.tile_pool(name="psO", bufs=2, space="PSUM") as pop,
                tc.tile_pool(name="psSK", bufs=1, space="PSUM") as pskp,
                tc.tile_pool(name="psT", bufs=1, space="PSUM") as pstp,
            ):
                w_sb = bigp.tile([128, KT * 1024], bf16, tag="w_sb")

                HTG = 2          # chunks per staged hidden group
                ht_tiles = {}

                def emit_ht(g):
                    t0i = g * HTG * CHUNK
                    ht = workp.tile([128, KT * HTG * CHUNK], bf16, tag="ht_blk",
                                    name=f"ht_blk_{g}")
                    for k in range(KT):
                        nc.sync.dma_start(
                            ht[:, k * HTG * CHUNK:(k + 1) * HTG * CHUNK],
                            hT_b[k * 128:(k + 1) * 128, t0i:t0i + HTG * CHUNK])
                    ht_tiles[g] = ht

                # interleave weight + first hidden loads so matmul k=0 can
                # start as soon as its two operand tiles have landed
                ht0 = workp.tile([128, KT * HTG * CHUNK], bf16, tag="ht_blk",
                                 name="ht_blk_0")
                for k in range(KT):
                    nc.sync.dma_start(w_sb[:, k * 1024:(k + 1) * 1024],
                                      w_pack[k * 128:(k + 1) * 128, :])
                    nc.sync.dma_start(
                        ht0[:, k * HTG * CHUNK:(k + 1) * HTG * CHUNK],
                        hT_b[k * 128:(k + 1) * 128, 0:HTG * CHUNK])
                ht_tiles[0] = ht0
                emit_ht(1)
                aT0 = None
                for tb in range(NTB):
                    if tb == 1:
                        # dense weights load: overlaps phase compute
                        for i in range(KT):
                            nc.sync.dma_start(wd_sb[:, i * HID:(i + 1) * HID],
                                              w_dense_sc[i * 128:(i + 1) * 128, :])
                    t0 = tb * TB * CHUNK
                    v_tb = ringp.tile([128, TB * HPC * D], bf16, tag="v_tb",
                                      name=f"v_tb_{tb}")
                    ks_tb = ringp.tile([128, TB * HPC * D], bf16, tag="ks_tb",
                                       name=f"ks_tb_{tb}")
                    gate_tb = ringp.tile([128, TB * HPC * D], bf16, tag="gate_tb",
                                         name=f"gate_tb_{tb}")

                    # ---------- phase 1: projections + norm + rope ----------
                    qk_tiles = []
                    for j in range(TB):
                        c = tb * TB + j
                        g, jj = c // HTG, c % HTG
                        ht_blk = ht_tiles[g]
                        psA = pap.tile([128, 512], f32, tag="psA")
                        psB = pbp.tile([128, 512], f32, tag="psB")
                        for k in range(KT):
                            ht_v = ht_blk[:, k * HTG * CHUNK + jj * 128:
                                          k * HTG * CHUNK + (jj + 1) * 128]
                            nc.tensor.matmul(psA[:], ht_v, w_sb[:, k * 1024:k * 1024 + 512],
                                             start=(k == 0), stop=(k == KT - 1))
                            nc.tensor.matmul(psB[:], ht_v, w_sb[:, k * 1024 + 512:(k + 1) * 1024],
                                             start=(k == 0), stop=(k == KT - 1))
                        if jj == HTG - 1:
                            ht_tiles.pop(g)
                            if g + 2 <= (T // CHUNK - 1) // HTG:
                                emit_ht(g + 2)

                        # early psum evacuation (frees banks for next tile's matmuls)
                        qk_raw = workp.tile([128, 512], f32, tag="qk_raw")
                        nc.vector.tensor_copy(qk_raw[:], psA[:])
                        cs = j * HPC * D
                        nc.vector.tensor_copy(v_tb[:, cs:cs + HPC * D], psB[:, 0:HPC * D])
                        g_raw = workp.tile([128, 256], f32, tag="g_raw")
                        nc.vector.tensor_copy(g_raw[:], psB[:, HPC * D:2 * HPC * D])

                        # RMS stats on raw q/k (per 128-block: q0 q1 k0 k1)
                        sumsq = workp.tile([128, 4], f32, tag="sumsq")
                        sq_scr = workp.tile([128, 128], bf16, tag="sq_scr")
                        for b in range(4):
                            nc.scalar.activation(sq_scr[:], qk_raw[:, b * 128:(b + 1) * 128],
                                                 AF.Square, bias=zero_sb[:, 0:1],
                                                 accum_out=sumsq[:, b:b + 1])
                        srt = workp.tile([128, 4], f32, tag="srt")
                        nc.scalar.activation(srt[:], sumsq[:], AF.Sqrt,
                                             scale=1.0 / D, bias=eps_sb[:, 0:1])
                        rstd = workp.tile([128, 4], f32, tag="rstd")
                        nc.vector.reciprocal(rstd[:], srt[:])

                        # sigmoid gate (from sbuf copy)
                        nc.scalar.activation(gate_tb[:, cs:cs + HPC * D],
                                             g_raw[:], AF.Sigmoid, bias=zero_sb[:, 0:1])

                        # rope on raw values
                        cos_t = workp.tile([128, 128], f32, tag="cos_t")
                        sin_t = workp.tile([128, 128], f32, tag="sin_t")
                        nc.sync.dma_start(cos_t[:], cos4[c * 128:(c + 1) * 128, :])
                        nc.sync.dma_start(sin_t[:], sin4[c * 128:(c + 1) * 128, :])
                        cosv = cos_t[:].rearrange("p (b x) -> p b x", x=32)
                        sinv = sin_t[:].rearrange("p (b x) -> p b x", x=32)
                        pav = qk_raw[:].rearrange("p (b x) -> p b x", x=128)
                        x1, x2 = pav[:, :, 0:32], pav[:, :, 32:64]
                        rp = workp.tile([128, 512], f32, tag="rp")
                        rpv = rp[:].rearrange("p (b x) -> p b x", x=128)
                        mA = workp.tile([128, 128], f32, tag="mA")
                        mB = workp.tile([128, 128], f32, tag="mB")
                        mAv = mA[:].rearrange("p (b x) -> p b x", x=32)
                        mBv = mB[:].rearrange("p (b x) -> p b x", x=32)
                        nc.vector.tensor_mul(mAv, x1, cosv)
                        nc.vector.tensor_mul(mBv, x2, sinv)
                        nc.vector.tensor_sub(rpv[:, :, 0:32], mAv, mBv)
                        mC = workp.tile([128, 128], f32, tag="mC")
                        mD = workp.tile([128, 128], f32, tag="mD")
                        mCv = mC[:].rearrange("p (b x) -> p b x", x=32)
                        mDv = mD[:].rearrange("p (b x) -> p b x", x=32)
                        nc.vector.tensor_mul(mCv, x2, cosv)
                        nc.vector.tensor_mul(mDv, x1, sinv)
                        nc.vector.tensor_add(rpv[:, :, 32:64], mCv, mDv)
                        nc.vector.tensor_copy(rpv[:, :, 64:128], pav[:, :, 64:128])
                        if apply_norm_w:
                            nc.vector.tensor_mul(rp[:], rp[:], normw_sb[:])

                        # scale by rstd, cast bf16 (DVE per-partition scalar)
                        qk_nat = workp.tile([128, 512], bf16, tag="qk_nat")
                        for b in range(4):
                            nc.vector.tensor_scalar_mul(
                                qk_nat[:, b * 128:(b + 1) * 128],
                                rp[:, b * 128:(b + 1) * 128], rstd[:, b:b + 1])

                        # k_scaled
                        for h in range(HPC):
                            nc.vector.tensor_scalar_mul(
                                ks_tb[:, cs + h * D:cs + (h + 1) * D],
                                qk_nat[:, 256 + h * 128:256 + (h + 1) * 128],
                                kdec_sb[:, h:h + 1])

                        # transpose q|k on the PE (SBUF->PSUM->SBUF, no DRAM trip)
                        psT = pstp.tile([128, 512], bf16, tag="psT")
                        for b in range(4):
                            nc.tensor.transpose(psT[:, b * 128:(b + 1) * 128],
                                                qk_nat[:, b * 128:(b + 1) * 128],
                                                ident_sb[:])
                        qkT = workp.tile([128, 512], bf16, tag="qkT", bufs=6,
                                         name=f"qkT_{c}")
                        nc.vector.tensor_copy(qkT[:], psT[:])
                        qk_tiles.append(qkT)

                    # ---------- phase 2: GLA scan ----------
                    for j in range(TB):
                        c = tb * TB + j
                        cs = j * HPC * D
                        sk_ps = pskp.tile([128, 512], f32, tag="sk")
                        st_ps = sk_ps[:, 0:256]
                        kv_ps = sk_ps[:, 256:512]
                        o_ps = pop.tile([128, HPC * D], f32, tag="o")
                        qkT = qk_tiles[j]
                        for h in range(HPC):
                            hh = h * 128
                            qTv = qkT[:, h * 128:(h + 1) * 128]
                            kTv = qkT[:, 256 + h * 128:256 + (h + 1) * 128]
                            vv = v_tb[:, cs + h * D:cs + (h + 1) * D]
                            ksv = ks_tb[:, cs + h * D:cs + (h + 1) * D]
                            # St[s,c] = k @ q^T
                            nc.tensor.matmul(st_ps[:, hh:hh + 128], kTv, qTv,
                                             start=True, stop=True)
                            At = workp.tile([128, 128], bf16, tag=f"At{h}",
                                            name=f"At{h}_{c}")
                            nc.vector.tensor_mul(At[:], st_ps[:, hh:hh + 128],
                                                 adect_sb[:, hh:hh + 128])
                            qs = workp.tile([128, 128], bf16, tag=f"qs{h}",
                                            name=f"qs{h}_{c}")
                            nc.vector.tensor_mul(qs[:], qTv, qdecb_sb[:, hh:hh + 128])
                            # o = At.T @ v + qs.T @ S
                            nc.tensor.matmul(o_ps[:, hh:hh + 128], At[:], vv,
                                             start=True, stop=False)
                            nc.tensor.matmul(o_ps[:, hh:hh + 128], qs[:], S_b[:, hh:hh + 128],
                                             start=False, stop=True)
                            # KV = ks.T @ v ; S = sdec*S + KV
                            nc.tensor.matmul(kv_ps[:, hh:hh + 128], ksv, vv,
                                             start=True, stop=True)
                            nc.vector.scalar_tensor_tensor(
                                S_sb[:, hh:hh + 128], S_sb[:, hh:hh + 128],
                                sdec_sb[:, h:h + 1], kv_ps[:, hh:hh + 128],
                                op0=ALU.mult, op1=ALU.add)
                            nc.vector.tensor_copy(S_b[:, hh:hh + 128], S_sb[:, hh:hh + 128])

                        # group rmsnorm (group == this core's 256 dims) + gate
                        gn_scr = workp.tile([128, HPC * D], bf16, tag="gn_scr")
                        gsumsq = workp.tile([128, 1], f32, tag="gsumsq")
                        nc.scalar.activation(gn_scr[:], o_ps[:], AF.Square,
                                             bias=zero_sb[:, 0:1], accum_out=gsumsq[:])
                        gsrt = workp.tile([128, 1], f32, tag="gsrt")
                        nc.scalar.activation(gsrt[:], gsumsq[:], AF.Sqrt,
                                             scale=1.0 / (HPC * D), bias=eps_sb[:, 0:1])
                        grstd = workp.tile([128, 1], f32, tag="grstd")
                        nc.vector.reciprocal(grstd[:], gsrt[:])
                        attn_t = workp.tile([128, HPC * D], bf16, tag="attn_t")
                        nc.vector.scalar_tensor_tensor(
                            attn_t[:], o_ps[:], grstd[:], gate_tb[:, cs:cs + HPC * D],
                            op0=ALU.mult, op1=ALU.mult)
                        seg = 0 if c < 32 else (1 if c < 48 else 2)
                        ch = c - SEG_CH[seg][0]
                        attn_dma = nc.sync.dma_start(
                            attn_seg[seg][ch * 128:(ch + 1) * 128, :], attn_t[:])
                        if c in (44, 57):
                            aT_anchor[0 if c == 44 else 1] = attn_dma

                    # segment collectives fire as soon as their chunks are done
                    if tb == 7:
                        nc.gpsimd.collective_compute(
                            "AllToAll", ALU.bypass,
                            replica_groups=[list(range(N_CORES))],
                            ins=[attn_seg[0].ap().opt()],
                            outs=[att_x[0].ap().opt()],
                        )
                    if tb == 11:
                        nc.gpsimd.collective_compute(
                            "AllToAll", ALU.bypass,
                            replica_groups=[list(range(N_CORES))],
                            ins=[attn_seg[1].ap().opt()],
                            outs=[att_x[1].ap().opt()],
                        )
                    # dense transposes issued mid-loop (after their collective
                    # is truly done, via anchors) so dense matmuls are ready
                    # the moment the phase loop drains
                    if tb == 13:
                        aT0 = emit_aT(0, anchor=aT_anchor[0])
                    if tb == 15:
                        aT1 = emit_aT(1, anchor=aT_anchor[1])

                # epilogue: final state out
                for h in range(HPC):
                    nc.sync.dma_start(state_out[h * D:(h + 1) * D, :],
                                      S_sb[:, h * D:(h + 1) * D])

            # last all-to-all (small: 2048 rows)
            nc.gpsimd.collective_compute(
                "AllToAll", ALU.bypass,
                replica_groups=[list(range(N_CORES))],
                ins=[attn_seg[2].ap().opt()],
                outs=[att_x[2].ap().opt()],
            )
            with tc.tile_pool(name="psDp", bufs=2, space="PSUM") as pdp:
                emit_dense(0, aT0, pdp)
                emit_dense(1, aT1, pdp)
                aT2 = emit_aT(2)
                emit_dense(2, aT2, pdp)

    nc.compile()
    return nc


_NC_CACHE: dict = {}


def _get_nc(apply_norm_w: bool):
    key = apply_norm_w
    if key not in _NC_CACHE:
        _NC_CACHE[key] = _build_nc(apply_norm_w)
    return _NC_CACHE[key]


def _host_constants(positions):
    gam = _slopes()  # [H] float64
    c = np.arange(CHUNK, dtype=np.float64)
    dscale = D ** -0.5
    dt_ = c[None, :] - c[:, None]  # [s, c]
    adect = (np.where(dt_ >= 0, np.exp(gam[:, None, None] * dt_[None]), 0.0) * dscale)
    qdec = np.exp(gam[:, None] * (c + 1.0)[None, :]) * dscale        # [H, c]
    kdec = np.exp(gam[:, None] * (CHUNK - 1.0 - c)[None, :])         # [H, s]
    sdec = np.exp(gam * CHUNK)                                       # [H]
    inv = 1.0 / (THETA ** (np.arange(0, RD, 2, dtype=np.float64) / RD))
    ang = positions.astype(np.float64)[:, None] * inv[None, :]       # [T, 32]
    cos = np.cos(ang).astype(np.float32)
    sin = np.sin(ang).astype(np.float32)
    cos4 = np.tile(cos, (1, 4)).astype(np.float32)                   # [T, 128]
    sin4 = np.tile(sin, (1, 4)).astype(np.float32)
    return (adect.astype(np.float32), qdec.astype(np.float32),
            kdec.astype(np.float32), sdec.astype(np.float32), cos4, sin4)


def kernel(positions, hidden_states, recurrent_state, w_qkv, w_g, w_dense,
           q_norm_w, k_norm_w, g_norm_w):
    positions = np.asarray(positions)
    hidden_states = np.asarray(hidden_states, dtype=np.float32)
    recurrent_state = np.asarray(recurrent_state, dtype=np.float32)
    w_qkv = np.asarray(w_qkv, dtype=np.float32)
    w_g = np.asarray(w_g, dtype=np.float32)
    w_dense = np.asarray(w_dense, dtype=np.float32)
    q_norm_w = np.asarray(q_norm_w, dtype=np.float32)
    k_norm_w = np.asarray(k_norm_w, dtype=np.float32)
    g_norm_w = np.asarray(g_norm_w, dtype=np.float32)

    apply_norm_w = not (np.all(q_norm_w == 1.0) and np.all(k_norm_w == 1.0))
    nc = _get_nc(apply_norm_w)

    adect, qdec, kdec, sdec, cos4, sin4 = _host_constants(positions)
    hT_b = np.ascontiguousarray(hidden_states.T).astype(BF16_NP)
    w_dense_sc = (w_dense * g_norm_w[:, None]).astype(BF16_NP)

    in_maps = []
    for core in range(N_CORES):
        heads = [core * HPC + i for i in range(HPC)]
        cols = []
        for h in heads:
            cols.append(w_qkv[:, h * D:(h + 1) * D])
        for h in heads:
            cols.append(w_qkv[:, H * D + h * D:H * D + (h + 1) * D])
        for h in heads:
            cols.append(w_qkv[:, 2 * H * D + h * D:2 * H * D + (h + 1) * D])
        cols.append(w_g[:, core * HPC * D:(core + 1) * HPC * D])
        w_pack = np.ascontiguousarray(np.concatenate(cols, axis=1)).astype(BF16_NP)

        adect_c = np.ascontiguousarray(
            adect[heads].reshape(HPC * CHUNK, CHUNK))
        # qdecb: [HPC*D, CHUNK], broadcast of qdec over d
        qdecb_c = np.ascontiguousarray(
            np.broadcast_to(qdec[heads][:, None, :], (HPC, D, CHUNK))
            .reshape(HPC * D, CHUNK))
        kdec_c = np.ascontiguousarray(kdec[heads].T)                  # [CHUNK, HPC]
        sdec_c = np.ascontiguousarray(
            np.broadcast_to(sdec[heads][None, :], (CHUNK, HPC))).astype(np.float32)
        state_c = np.ascontiguousarray(
            recurrent_state[heads].reshape(HPC * D, D))

        m = {
            "hT_b": hT_b,
            "w_pack": w_pack,
            "w_dense_sc": w_dense_sc,
            "cos4": cos4,
            "sin4": sin4,
            "adect": adect_c,
            "qdecb": qdecb_c,
            "kdec": kdec_c,
            "sdec": sdec_c,
            "state_in": state_c,
            "ident": np.eye(128, dtype=np.float32).astype(BF16_NP),
        }
        if apply_norm_w:
            nw = np.concatenate([q_norm_w, q_norm_w, k_norm_w, k_norm_w])
            m["normw"] = np.ascontiguousarray(
                np.broadcast_to(nw[None, :], (CHUNK, 512))).astype(np.float32)
        in_maps.append(m)

    global _last_in_maps
    _last_in_maps = in_maps
    res = bass_utils.run_bass_kernel_spmd(nc, in_maps, core_ids=list(range(N_CORES)))
    results = res.results

    # out_slice[c] = [t in [512c,512c+512) ; [4096+256c,+256) ; [6144+256c,+256)]
    out = np.empty((T, HID), dtype=np.float32)
    for c in range(N_CORES):
        sl = results[c]["out_slice"]
        out[512 * c:512 * (c + 1)] = sl[0:512]
        out[4096 + 256 * c:4096 + 256 * (c + 1)] = sl[512:768]
        out[6144 + 256 * c:6144 + 256 * (c + 1)] = sl[768:1024]
    new_state = np.concatenate(
        [results[c]["state_out"] for c in range(N_CORES)], axis=0
    ).reshape(H, D, D).astype(np.float32)
    return out, new_state


# revision 34
# speedup vs baseline: 1.0852x; 1.0852x over previous
"""Trainium2 Bass kernel for BailingMoeV2.5 linear attention (simple GLA).

Sharding: tensor-parallel over heads. 8 cores x 2 heads each.
  - qkv + gate projections: per-core output-column shards, transposed-hidden
    (precomputed on host, bf16) as the stationary matmul operand.
  - q/k RMSNorm + partial RoPE fused into the projection epilogue.
  - chunked simple-GLA scan (chunk=128), sequential over 64 chunks,
    embarrassingly parallel over heads; fp32 state, bf16 matmuls.
  - GroupRMSNorm group == the core's own 2 heads (local); sigmoid gate.
  - AllGather of bf16 attention output (4.2MB/core), then each core computes
    a 256-column slice of the dense projection (g_norm_w folded into w_dense).
All matmuls bf16 with fp32 PSUM accumulation.
"""
import math
import numpy as np
import ml_dtypes

import concourse.bass as bass
import concourse.bacc as bacc
import concourse.mybir as mybir
import concourse.tile as tile
import concourse.bass_utils as bass_utils

BF16_NP = ml_dtypes.bfloat16
DT = mybir.dt
AF = mybir.ActivationFunctionType
ALU = mybir.AluOpType

T, H, D, HID = 8192, 16, 128, 2048
RD = 64
THETA = 10000.0
EPS = 1e-6
LAYER_IDX, N_LAYERS = 12, 32
CHUNK = 128
NCH = T // CHUNK            # 64 chunks
N_CORES = 8
HPC = H // N_CORES          # 2 heads per core
JS = HID // N_CORES         # 256 output cols per core in dense
TB = 4                      # chunks per t-block (512 t per block)
NTB = NCH // TB             # 16 t-blocks
KT = HID // 128             # 16 k-tiles


def _slopes():
    start = 2.0 ** (-(2.0 ** (-(math.log2(H) - 3))))
    base = np.array([start * start ** i for i in range(H)], dtype=np.float64)
    return base * (-(1.0 - LAYER_IDX / (N_LAYERS - 1) + 1e-5))


def _build_nc(apply_norm_w: bool):
    nc = bacc.Bacc("TRN2", target_bir_lowering=False, debug=False,
                   enable_asserts=False, num_devices=N_CORES)

    f32, bf16 = DT.float32, DT.bfloat16

    # ---- I/O ----
    hT_b = nc.dram_tensor("hT_b", [HID, T], bf16, kind="ExternalInput")
    w_pack = nc.dram_tensor("w_pack", [HID, 4 * HPC * D], bf16, kind="ExternalInput")
    w_dense_sc = nc.dram_tensor("w_dense_sc", [HID, HID], bf16, kind="ExternalInput")
    cos4 = nc.dram_tensor("cos4", [T, 128], f32, kind="ExternalInput")
    sin4 = nc.dram_tensor("sin4", [T, 128], f32, kind="ExternalInput")
    adect_in = nc.dram_tensor("adect", [HPC * CHUNK, CHUNK], f32, kind="ExternalInput")
    qdecb_in = nc.dram_tensor("qdecb", [HPC * D, CHUNK], f32, kind="ExternalInput")
    kdec_in = nc.dram_tensor("kdec", [CHUNK, HPC], f32, kind="ExternalInput")
    sdec_in = nc.dram_tensor("sdec", [CHUNK, HPC], f32, kind="ExternalInput")
    state_in = nc.dram_tensor("state_in", [HPC * D, D], f32, kind="ExternalInput")
    ident_in = nc.dram_tensor("ident", [128, 128], bf16, kind="ExternalInput")
    if apply_norm_w:
        normw_in = nc.dram_tensor("normw", [CHUNK, 512], f32, kind="ExternalInput")

    TS = T // N_CORES  # 1024: t-slice per core after all-to-all
    out_slice = nc.dram_tensor("out_slice", [TS, HID], f32, kind="ExternalOutput")
    state_out = nc.dram_tensor("state_out", [HPC * D, D], f32, kind="ExternalOutput")

    # ---- internal DRAM ----
    # three t-segments, one all-to-all each (separate tensors so collective
    # deps don't serialize the segments): chunks [0,32), [32,48), [48,64)
    SEG_CH = [(0, 32), (32, 48), (48, 64)]     # chunk ranges
    SEG_ROWS = [(hi - lo) * CHUNK for lo, hi in SEG_CH]
    SEG_SHARD = [r // N_CORES for r in SEG_ROWS]  # per-rank rows: 512/256/256
    attn_seg = [nc.dram_tensor(f"attn_loc{i}", [SEG_ROWS[i], HPC * D], bf16)
                for i in range(3)]
    att_x = [nc.dram_tensor(f"att_x{i}", [SEG_ROWS[i], HPC * D], bf16)
             for i in range(3)]

    with tile.TileContext(nc) as tc:
        with (
            tc.tile_pool(name="const", bufs=1) as cpool,
            tc.tile_pool(name="densew", bufs=1) as dwp,
            tc.tile_pool(name="densework", bufs=1) as dwork,
            tc.tile_pool(name="psT", bufs=1, space="PSUM") as pstp,
        ):
            S_sb = cpool.tile([128, HPC * D], f32, tag="S_sb")
            S_b = cpool.tile([128, HPC * D], bf16, tag="S_b")
            adect_sb = cpool.tile([128, HPC * CHUNK], f32, tag="adect")
            qdecb_sb = cpool.tile([128, HPC * CHUNK], f32, tag="qdecb")
            kdec_sb = cpool.tile([128, HPC], f32, tag="kdec")
            sdec_sb = cpool.tile([128, HPC], f32, tag="sdec")
            if apply_norm_w:
                normw_sb = cpool.tile([128, 512], f32, tag="normw")
                nc.sync.dma_start(normw_sb[:], normw_in[:, :])
            ident_sb = cpool.tile([128, 128], bf16, tag="ident")
            nc.sync.dma_start(ident_sb[:], ident_in[:, :])
            eps_sb = cpool.tile([128, 1], f32, tag="eps")
            zero_sb = cpool.tile([128, 1], f32, tag="zero")
            nc.vector.memset(eps_sb[:], EPS)
            nc.vector.memset(zero_sb[:], 0.0)

            for h in range(HPC):
                nc.sync.dma_start(S_sb[:, h * D:(h + 1) * D],
                                  state_in[h * D:(h + 1) * D, :])
                nc.sync.dma_start(adect_sb[:, h * CHUNK:(h + 1) * CHUNK],
                                  adect_in[h * CHUNK:(h + 1) * CHUNK, :])
                nc.sync.dma_start(qdecb_sb[:, h * CHUNK:(h + 1) * CHUNK],
                                  qdecb_in[h * D:(h + 1) * D, :])
            nc.sync.dma_start(kdec_sb[:], kdec_in[:, :])
            nc.sync.dma_start(sdec_sb[:], sdec_in[:, :])
            nc.vector.tensor_copy(S_b[:], S_sb[:])

            # dense weights: tile at top-level scope, loads emitted later
            wd_sb = dwp.tile([128, KT * HID], bf16, tag="wd_sb")

            aT_anchor = [None, None, None]

            def emit_nat(seg, anchor=None):
                # plain (cheap, non-xbar) loads of the all-to-all result in
                # natural [t, i] layout; anchored so the scheduler doesn't
                # hoist them ahead of the collective (head-of-line blocking)
                rows = SEG_SHARD[seg]
                ntt = rows // 128
                nat = dwork.tile([128, 32 * 256], bf16, tag="nat",
                                 bufs=1, name=f"nat_{seg}")
                for tt in range(ntt):
                    for j8 in range(N_CORES):
                        dm = nc.sync.dma_start(
                            nat[:, (tt * 8 + j8) * 256:(tt * 8 + j8 + 1) * 256],
                            att_x[seg][j8 * rows + tt * 128:
                                       j8 * rows + (tt + 1) * 128, :])
                        if anchor is not None:
                            tile.add_dep_helper(dm.ins, anchor.ins,
                                                reason="nat after anchor")
                return nat

            def emit_aTpe(seg, nat):
                # PE-transpose the natural tiles into [i, t] lhsT layout
                rows = SEG_SHARD[seg]
                ntt = rows // 128
                aT = dwork.tile([128, KT * rows], bf16, tag=f"aT{seg}", bufs=1,
                                name=f"aT_{seg}")
                aTv = aT[:].rearrange("p (i r) -> p i r", r=rows)
                for tt in range(ntt):
                    for r in range(4):
                        psTt = pstp.tile([128, 512], bf16, tag="psT",
                                         name=f"psTd_{seg}_{tt}_{r}")
                        for q in range(4):
                            idx = r * 4 + q
                            nc.tensor.transpose(
                                psTt[:, q * 128:(q + 1) * 128],
                                nat[:, (tt * 8 + idx // 2) * 256 + (idx % 2) * 128:
                                    (tt * 8 + idx // 2) * 256 + (idx % 2 + 1) * 128],
                                ident_sb[:])
                        nc.vector.tensor_copy(
                            aTv[:, 4 * r:4 * (r + 1), tt * 128:(tt + 1) * 128],
                            psTt[:].rearrange("p (i r) -> p i r", r=128))
                return aT

            def emit_dense(seg, aT, pdp):
                rows = SEG_SHARD[seg]
                rbase = sum(SEG_SHARD[:seg])
                for tt in range(rows // 128):
                    for jb in range(4):
                        psD = pdp.tile([128, 512], f32, tag="psD",
                                       name=f"psD_{seg}_{tt}_{jb}")
                        for i in range(KT):
                            lhs = aT[:, i * rows + tt * 128: i * rows + (tt + 1) * 128]
                            nc.tensor.matmul(
                                psD[:], lhs,
                                wd_sb[:, i * HID + jb * 512: i * HID + (jb + 1) * 512],
                                start=(i == 0), stop=(i == KT - 1))
                        oc = dwork.tile([128, 512], f32, tag="oc", bufs=2,
                                        name=f"oc_{seg}_{tt}_{jb}")
                        nc.vector.tensor_copy(oc[:], psD[:])
                        r = rbase + tt * 128
                        nc.sync.dma_start(
                            out_slice[r:r + 128, jb * 512:(jb + 1) * 512], oc[:])

            with (
                tc.tile_pool(name="big", bufs=1) as bigp,
                tc.tile_pool(name="ring", bufs=2) as ringp,
                tc.tile_pool(name="work", bufs=2) as workp,
                tc.tile_pool(name="psA", bufs=2, space="PSUM") as pap,
                tc.tile_pool(name="psB", bufs=2, space="PSUM") as pbp,
                tc.tile_pool(name="psO", bufs=2, space="PSUM") as pop,
                tc.tile_pool(name="psSK", bufs=1, space="PSUM") as pskp,
            ):
                w_sb = bigp.tile([128, KT * 1024], bf16, tag="w_sb")

                HTG = 2          # chunks per staged hidden group
                ht_tiles = {}

                def emit_ht(g):
                    t0i = g * HTG * CHUNK
                    ht = workp.tile([128, KT * HTG * CHUNK], bf16, tag="ht_blk",
                                    name=f"ht_blk_{g}")
                    for k in range(KT):
                        nc.sync.dma_start(
                            ht[:, k * HTG * CHUNK:(k + 1) * HTG * CHUNK],
                            hT_b[k * 128:(k + 1) * 128, t0i:t0i + HTG * CHUNK])
                    ht_tiles[g] = ht

                # interleave weight + first hidden loads so matmul k=0 can
                # start as soon as its two operand tiles have landed
                ht0 = workp.tile([128, KT * HTG * CHUNK], bf16, tag="ht_blk",
                                 name="ht_blk_0")
                for k in range(KT):
                    nc.sync.dma_start(w_sb[:, k * 1024:(k + 1) * 1024],
                                      w_pack[k * 128:(k + 1) * 128, :])
                    nc.sync.dma_start(
                        ht0[:, k * HTG * CHUNK:(k + 1) * HTG * CHUNK],
                        hT_b[k * 128:(k + 1) * 128, 0:HTG * CHUNK])
                ht_tiles[0] = ht0
                emit_ht(1)
                aT0 = None
                for tb in range(NTB):
                    if tb == 5:
                        # dense weights load: overlaps phase compute
                        for i in range(KT):
                            nc.sync.dma_start(wd_sb[:, i * HID:(i + 1) * HID],
                                              w_dense_sc[i * 128:(i + 1) * 128, :])
                    t0 = tb * TB * CHUNK
                    v_tb = ringp.tile([128, TB * HPC * D], bf16, tag="v_tb",
                                      name=f"v_tb_{tb}")
                    ks_tb = ringp.tile([128, TB * HPC * D], bf16, tag="ks_tb",
                                       name=f"ks_tb_{tb}")
                    gate_tb = ringp.tile([128, TB * HPC * D], bf16, tag="gate_tb",
                                         name=f"gate_tb_{tb}")

                    # ---------- phase 1: projections + norm + rope ----------
                    qk_tiles = []
                    for j in range(TB):
                        c = tb * TB + j
                        g, jj = c // HTG, c % HTG
                        ht_blk = ht_tiles[g]
                        psA = pap.tile([128, 512], f32, tag="psA")
                        psB = pbp.tile([128, 512], f32, tag="psB")
                        for k in range(KT):
                            ht_v = ht_blk[:, k * HTG * CHUNK + jj * 128:
                                          k * HTG * CHUNK + (jj + 1) * 128]
                            nc.tensor.matmul(psA[:], ht_v, w_sb[:, k * 1024:k * 1024 + 512],
                                             start=(k == 0), stop=(k == KT - 1))
                            nc.tensor.matmul(psB[:], ht_v, w_sb[:, k * 1024 + 512:(k + 1) * 1024],
                                             start=(k == 0), stop=(k == KT - 1))
                        if jj == HTG - 1:
                            ht_tiles.pop(g)
                            if g + 2 <= (T // CHUNK - 1) // HTG:
                                emit_ht(g + 2)

                        # early psum evacuation (frees banks for next tile's matmuls)
                        qk_raw = workp.tile([128, 512], f32, tag="qk_raw")
                        nc.vector.tensor_copy(qk_raw[:], psA[:])
                        cs = j * HPC * D
                        nc.vector.tensor_copy(v_tb[:, cs:cs + HPC * D], psB[:, 0:HPC * D])
                        g_raw = workp.tile([128, 256], f32, tag="g_raw")
                        nc.vector.tensor_copy(g_raw[:], psB[:, HPC * D:2 * HPC * D])

                        # RMS stats on raw q/k (per 128-block: q0 q1 k0 k1)
                        sumsq = workp.tile([128, 4], f32, tag="sumsq")
                        sq_scr = workp.tile([128, 128], bf16, tag="sq_scr")
                        for b in range(4):
                            nc.scalar.activation(sq_scr[:], qk_raw[:, b * 128:(b + 1) * 128],
                                                 AF.Square, bias=zero_sb[:, 0:1],
                                                 accum_out=sumsq[:, b:b + 1])
                        srt = workp.tile([128, 4], f32, tag="srt")
                        nc.scalar.activation(srt[:], sumsq[:], AF.Sqrt,
                                             scale=1.0 / D, bias=eps_sb[:, 0:1])
                        rstd = workp.tile([128, 4], f32, tag="rstd")
                        nc.vector.reciprocal(rstd[:], srt[:])

                        # sigmoid gate (from sbuf copy)
                        nc.scalar.activation(gate_tb[:, cs:cs + HPC * D],
                                             g_raw[:], AF.Sigmoid, bias=zero_sb[:, 0:1])

                        # rope on raw values
                        cos_t = workp.tile([128, 128], f32, tag="cos_t")
                        sin_t = workp.tile([128, 128], f32, tag="sin_t")
                        nc.sync.dma_start(cos_t[:], cos4[c * 128:(c + 1) * 128, :])
                        nc.sync.dma_start(sin_t[:], sin4[c * 128:(c + 1) * 128, :])
                        cosv = cos_t[:].rearrange("p (b x) -> p b x", x=32)
                        sinv = sin_t[:].rearrange("p (b x) -> p b x", x=32)
                        pav = qk_raw[:].rearrange("p (b x) -> p b x", x=128)
                        x1, x2 = pav[:, :, 0:32], pav[:, :, 32:64]
                        rp = workp.tile([128, 512], f32, tag="rp")
                        rpv = rp[:].rearrange("p (b x) -> p b x", x=128)
                        mA = workp.tile([128, 128], f32, tag="mA")
                        mB = workp.tile([128, 128], f32, tag="mB")
                        mAv = mA[:].rearrange("p (b x) -> p b x", x=32)
                        mBv = mB[:].rearrange("p (b x) -> p b x", x=32)
                        nc.vector.tensor_mul(mAv, x1, cosv)
                        nc.vector.tensor_mul(mBv, x2, sinv)
                        nc.vector.tensor_sub(rpv[:, :, 0:32], mAv, mBv)
                        mC = workp.tile([128, 128], f32, tag="mC")
                        mD = workp.tile([128, 128], f32, tag="mD")
                        mCv = mC[:].rearrange("p (b x) -> p b x", x=32)
                        mDv = mD[:].rearrange("p (b x) -> p b x", x=32)
                        nc.vector.tensor_mul(mCv, x2, cosv)
                        nc.vector.tensor_mul(mDv, x1, sinv)
                        nc.vector.tensor_add(rpv[:, :, 32:64], mCv, mDv)
                        nc.vector.tensor_copy(rpv[:, :, 64:128], pav[:, :, 64:128])
                        if apply_norm_w:
                            nc.vector.tensor_mul(rp[:], rp[:], normw_sb[:])

                        # scale by rstd, cast bf16 (DVE per-partition scalar)
                        qk_nat = workp.tile([128, 512], bf16, tag="qk_nat")
                        for b in range(4):
                            nc.vector.tensor_scalar_mul(
                                qk_nat[:, b * 128:(b + 1) * 128],
                                rp[:, b * 128:(b + 1) * 128], rstd[:, b:b + 1])

                        # k_scaled
                        for h in range(HPC):
                            nc.vector.tensor_scalar_mul(
                                ks_tb[:, cs + h * D:cs + (h + 1) * D],
                                qk_nat[:, 256 + h * 128:256 + (h + 1) * 128],
                                kdec_sb[:, h:h + 1])

                        # transpose q|k on the PE (SBUF->PSUM->SBUF, no DRAM trip)
                        psT = pstp.tile([128, 512], bf16, tag="psT")
                        for b in range(4):
                            nc.tensor.transpose(psT[:, b * 128:(b + 1) * 128],
                                                qk_nat[:, b * 128:(b + 1) * 128],
                                                ident_sb[:])
                        qkT = workp.tile([128, 512], bf16, tag="qkT", bufs=5,
                                         name=f"qkT_{c}")
                        nc.vector.tensor_copy(qkT[:], psT[:])
                        qk_tiles.append(qkT)

                    # ---------- phase 2: GLA scan ----------
                    for j in range(TB):
                        c = tb * TB + j
                        cs = j * HPC * D
                        sk_ps = pskp.tile([128, 512], f32, tag="sk")
                        st_ps = sk_ps[:, 0:256]
                        kv_ps = sk_ps[:, 256:512]
                        o_ps = pop.tile([128, HPC * D], f32, tag="o")
                        qkT = qk_tiles[j]
                        for h in range(HPC):
                            hh = h * 128
                            qTv = qkT[:, h * 128:(h + 1) * 128]
                            kTv = qkT[:, 256 + h * 128:256 + (h + 1) * 128]
                            vv = v_tb[:, cs + h * D:cs + (h + 1) * D]
                            ksv = ks_tb[:, cs + h * D:cs + (h + 1) * D]
                            # St[s,c] = k @ q^T
                            nc.tensor.matmul(st_ps[:, hh:hh + 128], kTv, qTv,
                                             start=True, stop=True)
                            At = workp.tile([128, 128], bf16, tag=f"At{h}",
                                            name=f"At{h}_{c}")
                            nc.vector.tensor_mul(At[:], st_ps[:, hh:hh + 128],
                                                 adect_sb[:, hh:hh + 128])
                            qs = workp.tile([128, 128], bf16, tag=f"qs{h}",
                                            name=f"qs{h}_{c}")
                            nc.vector.tensor_mul(qs[:], qTv, qdecb_sb[:, hh:hh + 128])
                            # o = At.T @ v + qs.T @ S
                            nc.tensor.matmul(o_ps[:, hh:hh + 128], At[:], vv,
                                             start=True, stop=False)
                            nc.tensor.matmul(o_ps[:, hh:hh + 128], qs[:], S_b[:, hh:hh + 128],
                                             start=False, stop=True)
                            # KV = ks.T @ v ; S = sdec*S + KV
                            nc.tensor.matmul(kv_ps[:, hh:hh + 128], ksv, vv,
                                             start=True, stop=True)
                            nc.vector.scalar_tensor_tensor(
                                S_sb[:, hh:hh + 128], S_sb[:, hh:hh + 128],
                                sdec_sb[:, h:h + 1], kv_ps[:, hh:hh + 128],
                                op0=ALU.mult, op1=ALU.add)
                            nc.vector.tensor_copy(S_b[:, hh:hh + 128], S_sb[:, hh:hh + 128])

                        # group rmsnorm (group == this core's 256 dims) + gate
                        gn_scr = workp.tile([128, HPC * D], bf16, tag="gn_scr")
                        gsumsq = workp.tile([128, 1], f32, tag="gsumsq")
                        nc.scalar.activation(gn_scr[:], o_ps[:], AF.Square,
                                             bias=zero_sb[:, 0:1], accum_out=gsumsq[:])
                        gsrt = workp.tile([128, 1], f32, tag="gsrt")
                        nc.scalar.activation(gsrt[:], gsumsq[:], AF.Sqrt,
                                             scale=1.0 / (HPC * D), bias=eps_sb[:, 0:1])
                        grstd = workp.tile([128, 1], f32, tag="grstd")
                        nc.vector.reciprocal(grstd[:], gsrt[:])
                        attn_t = workp.tile([128, HPC * D], bf16, tag="attn_t")
                        nc.vector.scalar_tensor_tensor(
                            attn_t[:], o_ps[:], grstd[:], gate_tb[:, cs:cs + HPC * D],
                            op0=ALU.mult, op1=ALU.mult)
                        seg = 0 if c < 32 else (1 if c < 48 else 2)
                        ch = c - SEG_CH[seg][0]
                        attn_dma = nc.sync.dma_start(
                            attn_seg[seg][ch * 128:(ch + 1) * 128, :], attn_t[:])
                        if c in (48, 56):
                            aT_anchor[0 if c == 48 else 1] = attn_dma

                    # segment collectives fire as soon as their chunks are done
                    if tb == 7:
                        nc.gpsimd.collective_compute(
                            "AllToAll", ALU.bypass,
                            replica_groups=[list(range(N_CORES))],
                            ins=[attn_seg[0].ap().opt()],
                            outs=[att_x[0].ap().opt()],
                        )
                    if tb == 11:
                        nc.gpsimd.collective_compute(
                            "AllToAll", ALU.bypass,
                            replica_groups=[list(range(N_CORES))],
                            ins=[attn_seg[1].ap().opt()],
                            outs=[att_x[1].ap().opt()],
                        )
                    # dense lhsT prep issued mid-loop (after their collective
                    # is truly done, via anchors) so dense matmuls are ready
                    # the moment the phase loop drains
                    if tb == 13:
                        nat0 = emit_nat(0, anchor=aT_anchor[0])
                        aT0 = emit_aTpe(0, nat0)
                    if tb == 15:
                        nat1 = emit_nat(1, anchor=aT_anchor[1])
                        aT1 = emit_aTpe(1, nat1)

                # epilogue: final state out
                for h in range(HPC):
                    nc.sync.dma_start(state_out[h * D:(h + 1) * D, :],
                                      S_sb[:, h * D:(h + 1) * D])

            # last all-to-all (small: 2048 rows)
            nc.gpsimd.collective_compute(
                "AllToAll", ALU.bypass,
                replica_groups=[list(range(N_CORES))],
                ins=[attn_seg[2].ap().opt()],
                outs=[att_x[2].ap().opt()],
            )
            with tc.tile_pool(name="psDp", bufs=2, space="PSUM") as pdp:
                nat2 = emit_nat(2)
                emit_dense(0, aT0, pdp)
                aT2 = emit_aTpe(2, nat2)
                emit_dense(1, aT1, pdp)
                emit_dense(2, aT2, pdp)

    nc.compile()
    return nc


_NC_CACHE: dict = {}


def _get_nc(apply_norm_w: bool):
    key = apply_norm_w
    if key not in _NC_CACHE:
        _NC_CACHE[key] = _build_nc(apply_norm_w)
    return _NC_CACHE[key]


def _host_constants(positions):
    gam = _slopes()  # [H] float64
    c = np.arange(CHUNK, dtype=np.float64)
    dscale = D ** -0.5
    dt_ = c[None, :] - c[:, None]  # [s, c]
    adect = (np.where(dt_ >= 0, np.exp(gam[:, None, None] * dt_[None]), 0.0) * dscale)
    qdec = np.exp(gam[:, None] * (c + 1.0)[None, :]) * dscale        # [H, c]
    kdec = np.exp(gam[:, None] * (CHUNK - 1.0 - c)[None, :])         # [H, s]
    sdec = np.exp(gam * CHUNK)                                       # [H]
    inv = 1.0 / (THETA ** (np.arange(0, RD, 2, dtype=np.float64) / RD))
    ang = positions.astype(np.float64)[:, None] * inv[None, :]       # [T, 32]
    cos = np.cos(ang).astype(np.float32)
    sin = np.sin(ang).astype(np.float32)
    cos4 = np.tile(cos, (1, 4)).astype(np.float32)                   # [T, 128]
    sin4 = np.tile(sin, (1, 4)).astype(np.float32)
    return (adect.astype(np.float32), qdec.astype(np.float32),
            kdec.astype(np.float32), sdec.astype(np.float32), cos4, sin4)


def kernel(positions, hidden_states, recurrent_state, w_qkv, w_g, w_dense,
           q_norm_w, k_norm_w, g_norm_w):
    positions = np.asarray(positions)
    hidden_states = np.asarray(hidden_states, dtype=np.float32)
    recurrent_state = np.asarray(recurrent_state, dtype=np.float32)
    w_qkv = np.asarray(w_qkv, dtype=np.float32)
    w_g = np.asarray(w_g, dtype=np.float32)
    w_dense = np.asarray(w_dense, dtype=np.float32)
    q_norm_w = np.asarray(q_norm_w, dtype=np.float32)
    k_norm_w = np.asarray(k_norm_w, dtype=np.float32)
    g_norm_w = np.asarray(g_norm_w, dtype=np.float32)

    apply_norm_w = not (np.all(q_norm_w == 1.0) and np.all(k_norm_w == 1.0))
    nc = _get_nc(apply_norm_w)

    adect, qdec, kdec, sdec, cos4, sin4 = _host_constants(positions)
    hT_b = np.ascontiguousarray(hidden_states.T).astype(BF16_NP)
    w_dense_sc = (w_dense * g_norm_w[:, None]).astype(BF16_NP)

    in_maps = []
    for core in range(N_CORES):
        heads = [core * HPC + i for i in range(HPC)]
        cols = []
        for h in heads:
            cols.append(w_qkv[:, h * D:(h + 1) * D])
        for h in heads:
            cols.append(w_qkv[:, H * D + h * D:H * D + (h + 1) * D])
        for h in heads:
            cols.append(w_qkv[:, 2 * H * D + h * D:2 * H * D + (h + 1) * D])
        cols.append(w_g[:, core * HPC * D:(core + 1) * HPC * D])
        w_pack = np.ascontiguousarray(np.concatenate(cols, axis=1)).astype(BF16_NP)

        adect_c = np.ascontiguousarray(
            adect[heads].reshape(HPC * CHUNK, CHUNK))
        # qdecb: [HPC*D, CHUNK], broadcast of qdec over d
        qdecb_c = np.ascontiguousarray(
            np.broadcast_to(qdec[heads][:, None, :], (HPC, D, CHUNK))
            .reshape(HPC * D, CHUNK))
        kdec_c = np.ascontiguousarray(kdec[heads].T)                  # [CHUNK, HPC]
        sdec_c = np.ascontiguousarray(
            np.broadcast_to(sdec[heads][None, :], (CHUNK, HPC))).astype(np.float32)
        state_c = np.ascontiguousarray(
            recurrent_state[heads].reshape(HPC * D, D))

        m = {
            "hT_b": hT_b,
            "w_pack": w_pack,
            "w_dense_sc": w_dense_sc,
            "cos4": cos4,
            "sin4": sin4,
            "adect": adect_c,
            "qdecb": qdecb_c,
            "kdec": kdec_c,
            "sdec": sdec_c,
            "state_in": state_c,
            "ident": np.eye(128, dtype=np.float32).astype(BF16_NP),
        }
        if apply_norm_w:
            nw = np.concatenate([q_norm_w, q_norm_w, k_norm_w, k_norm_w])
            m["normw"] = np.ascontiguousarray(
                np.broadcast_to(nw[None, :], (CHUNK, 512))).astype(np.float32)
        in_maps.append(m)

    global _last_in_maps
    _last_in_maps = in_maps
    res = bass_utils.run_bass_kernel_spmd(nc, in_maps, core_ids=list(range(N_CORES)))
    results = res.results

    # out_slice[c] = [t in [512c,512c+512) ; [4096+256c,+256) ; [6144+256c,+256)]
    out = np.empty((T, HID), dtype=np.float32)
    for c in range(N_CORES):
        sl = results[c]["out_slice"]
        out[512 * c:512 * (c + 1)] = sl[0:512]
        out[4096 + 256 * c:4096 + 256 * (c + 1)] = sl[512:768]
        out[6144 + 256 * c:6144 + 256 * (c + 1)] = sl[768:1024]
    new_state = np.concatenate(
        [results[c]["state_out"] for c in range(N_CORES)], axis=0
    ).reshape(H, D, D).astype(np.float32)
    return out, new_state


# revision 37
# speedup vs baseline: 1.0911x; 1.0054x over previous
"""Trainium2 Bass kernel for BailingMoeV2.5 linear attention (simple GLA).

Sharding: tensor-parallel over heads. 8 cores x 2 heads each.
  - qkv + gate projections: per-core output-column shards, transposed-hidden
    (precomputed on host, bf16) as the stationary matmul operand.
  - q/k RMSNorm + partial RoPE fused into the projection epilogue.
  - chunked simple-GLA scan (chunk=128), sequential over 64 chunks,
    embarrassingly parallel over heads; fp32 state, bf16 matmuls.
  - GroupRMSNorm group == the core's own 2 heads (local); sigmoid gate.
  - AllGather of bf16 attention output (4.2MB/core), then each core computes
    a 256-column slice of the dense projection (g_norm_w folded into w_dense).
All matmuls bf16 with fp32 PSUM accumulation.
"""
import math
import numpy as np
import ml_dtypes

import concourse.bass as bass
import concourse.bacc as bacc
import concourse.mybir as mybir
import concourse.tile as tile
import concourse.bass_utils as bass_utils

BF16_NP = ml_dtypes.bfloat16
DT = mybir.dt
AF = mybir.ActivationFunctionType
ALU = mybir.AluOpType

T, H, D, HID = 8192, 16, 128, 2048
RD = 64
THETA = 10000.0
EPS = 1e-6
LAYER_IDX, N_LAYERS = 12, 32
CHUNK = 128
NCH = T // CHUNK            # 64 chunks
N_CORES = 8
HPC = H // N_CORES          # 2 heads per core
JS = HID // N_CORES         # 256 output cols per core in dense
TB = 4                      # chunks per t-block (512 t per block)
NTB = NCH // TB             # 16 t-blocks
KT = HID // 128             # 16 k-tiles


def _slopes():
    start = 2.0 ** (-(2.0 ** (-(math.log2(H) - 3))))
    base = np.array([start * start ** i for i in range(H)], dtype=np.float64)
    return base * (-(1.0 - LAYER_IDX / (N_LAYERS - 1) + 1e-5))


def _build_nc(apply_norm_w: bool):
    nc = bacc.Bacc("TRN2", target_bir_lowering=False, debug=False,
                   enable_asserts=False, num_devices=N_CORES)

    f32, bf16 = DT.float32, DT.bfloat16

    # ---- I/O ----
    hT_b = nc.dram_tensor("hT_b", [HID, T], bf16, kind="ExternalInput")
    w_pack = nc.dram_tensor("w_pack", [HID, 4 * HPC * D], bf16, kind="ExternalInput")
    w_dense_sc = nc.dram_tensor("w_dense_sc", [HID, HID], bf16, kind="ExternalInput")
    cossin = nc.dram_tensor("cossin", [T, 256], f32, kind="ExternalInput")
    adect_in = nc.dram_tensor("adect", [HPC * CHUNK, CHUNK], f32, kind="ExternalInput")
    qdecb_in = nc.dram_tensor("qdecb", [HPC * D, CHUNK], f32, kind="ExternalInput")
    kdec_in = nc.dram_tensor("kdec", [CHUNK, HPC], f32, kind="ExternalInput")
    sdec_in = nc.dram_tensor("sdec", [CHUNK, HPC], f32, kind="ExternalInput")
    state_in = nc.dram_tensor("state_in", [HPC * D, D], f32, kind="ExternalInput")
    ident_in = nc.dram_tensor("ident", [128, 128], bf16, kind="ExternalInput")
    if apply_norm_w:
        normw_in = nc.dram_tensor("normw", [CHUNK, 512], f32, kind="ExternalInput")

    TS = T // N_CORES  # 1024: t-slice per core after all-to-all
    out_slice = nc.dram_tensor("out_slice", [TS, HID], f32, kind="ExternalOutput")
    state_out = nc.dram_tensor("state_out", [HPC * D, D], f32, kind="ExternalOutput")

    # ---- internal DRAM ----
    # three t-segments, one all-to-all each (separate tensors so collective
    # deps don't serialize the segments): chunks [0,32), [32,48), [48,64)
    SEG_CH = [(0, 32), (32, 48), (48, 64)]     # chunk ranges
    SEG_ROWS = [(hi - lo) * CHUNK for lo, hi in SEG_CH]
    SEG_SHARD = [r // N_CORES for r in SEG_ROWS]  # per-rank rows: 512/256/256
    attn_seg = [nc.dram_tensor(f"attn_loc{i}", [SEG_ROWS[i], HPC * D], bf16)
                for i in range(3)]
    att_x = [nc.dram_tensor(f"att_x{i}", [SEG_ROWS[i], HPC * D], bf16)
             for i in range(3)]

    with tile.TileContext(nc) as tc:
        with (
            tc.tile_pool(name="const", bufs=1) as cpool,
            tc.tile_pool(name="densew", bufs=1) as dwp,
            tc.tile_pool(name="densework", bufs=1) as dwork,
            tc.tile_pool(name="psT", bufs=1, space="PSUM") as pstp,
        ):
            S_sb = cpool.tile([128, HPC * D], f32, tag="S_sb")
            S_b = cpool.tile([128, HPC * D], bf16, tag="S_b")
            adect_sb = cpool.tile([128, HPC * CHUNK], f32, tag="adect")
            qdecb_sb = cpool.tile([128, HPC * CHUNK], f32, tag="qdecb")
            kdec_sb = cpool.tile([128, HPC], f32, tag="kdec")
            sdec_sb = cpool.tile([128, HPC], f32, tag="sdec")
            if apply_norm_w:
                normw_sb = cpool.tile([128, 512], f32, tag="normw")
                nc.sync.dma_start(normw_sb[:], normw_in[:, :])
            ident_sb = cpool.tile([128, 128], bf16, tag="ident")
            nc.sync.dma_start(ident_sb[:], ident_in[:, :])
            eps_sb = cpool.tile([128, 1], f32, tag="eps")
            zero_sb = cpool.tile([128, 1], f32, tag="zero")
            nc.vector.memset(eps_sb[:], EPS)
            nc.vector.memset(zero_sb[:], 0.0)

            for h in range(HPC):
                nc.sync.dma_start(S_sb[:, h * D:(h + 1) * D],
                                  state_in[h * D:(h + 1) * D, :])
                nc.sync.dma_start(adect_sb[:, h * CHUNK:(h + 1) * CHUNK],
                                  adect_in[h * CHUNK:(h + 1) * CHUNK, :])
                nc.sync.dma_start(qdecb_sb[:, h * CHUNK:(h + 1) * CHUNK],
                                  qdecb_in[h * D:(h + 1) * D, :])
            nc.sync.dma_start(kdec_sb[:], kdec_in[:, :])
            nc.sync.dma_start(sdec_sb[:], sdec_in[:, :])
            nc.vector.tensor_copy(S_b[:], S_sb[:])

            # dense weights: tile at top-level scope, loads emitted later
            wd_sb = dwp.tile([128, KT * HID], bf16, tag="wd_sb")

            aT_anchor = [None, None, None]

            def emit_nat(seg, anchor=None):
                # plain (cheap, non-xbar) loads of the all-to-all result in
                # natural [t, i] layout; anchored so the scheduler doesn't
                # hoist them ahead of the collective (head-of-line blocking)
                rows = SEG_SHARD[seg]
                ntt = rows // 128
                nat = dwork.tile([128, 32 * 256], bf16, tag="nat",
                                 bufs=1, name=f"nat_{seg}")
                natv = nat[:].rearrange("p (b i) -> p b i", i=256)
                xv = att_x[seg].ap().rearrange("(j8 r p) i -> p j8 r i",
                                               p=128, r=ntt)
                for tt in range(ntt):
                    dm = nc.sync.dma_start(
                        natv[:, tt * 8:(tt + 1) * 8, :],
                        xv[:, :, tt, :])
                    if anchor is not None:
                        tile.add_dep_helper(dm.ins, anchor.ins,
                                            reason="nat after anchor")
                return nat

            def emit_aTpe(seg, nat):
                # PE-transpose the natural tiles into [i, t] lhsT layout
                rows = SEG_SHARD[seg]
                ntt = rows // 128
                aT = dwork.tile([128, KT * rows], bf16, tag=f"aT{seg}", bufs=1,
                                name=f"aT_{seg}")
                aTv = aT[:].rearrange("p (i r) -> p i r", r=rows)
                for tt in range(ntt):
                    for r in range(4):
                        psTt = pstp.tile([128, 512], bf16, tag="psT",
                                         name=f"psTd_{seg}_{tt}_{r}")
                        for q in range(4):
                            idx = r * 4 + q
                            nc.tensor.transpose(
                                psTt[:, q * 128:(q + 1) * 128],
                                nat[:, (tt * 8 + idx // 2) * 256 + (idx % 2) * 128:
                                    (tt * 8 + idx // 2) * 256 + (idx % 2 + 1) * 128],
                                ident_sb[:])
                        nc.vector.tensor_copy(
                            aTv[:, 4 * r:4 * (r + 1), tt * 128:(tt + 1) * 128],
                            psTt[:].rearrange("p (i r) -> p i r", r=128))
                return aT

            def emit_dense(seg, aT, pdp):
                rows = SEG_SHARD[seg]
                rbase = sum(SEG_SHARD[:seg])
                for tt in range(rows // 128):
                    for jb in range(4):
                        psD = pdp.tile([128, 512], f32, tag="psD",
                                       name=f"psD_{seg}_{tt}_{jb}")
                        for i in range(KT):
                            lhs = aT[:, i * rows + tt * 128: i * rows + (tt + 1) * 128]
                            nc.tensor.matmul(
                                psD[:], lhs,
                                wd_sb[:, i * HID + jb * 512: i * HID + (jb + 1) * 512],
                                start=(i == 0), stop=(i == KT - 1))
                        oc = dwork.tile([128, 512], f32, tag="oc", bufs=1,
                                        name=f"oc_{seg}_{tt}_{jb}")
                        nc.vector.tensor_copy(oc[:], psD[:])
                        r = rbase + tt * 128
                        nc.sync.dma_start(
                            out_slice[r:r + 128, jb * 512:(jb + 1) * 512], oc[:])

            with (
                tc.tile_pool(name="big", bufs=1) as bigp,
                tc.tile_pool(name="ring", bufs=2) as ringp,
                tc.tile_pool(name="work", bufs=2) as workp,
                tc.tile_pool(name="psA", bufs=2, space="PSUM") as pap,
                tc.tile_pool(name="psB", bufs=2, space="PSUM") as pbp,
                tc.tile_pool(name="psO", bufs=2, space="PSUM") as pop,
                tc.tile_pool(name="psSK", bufs=1, space="PSUM") as pskp,
            ):
                w_sb = bigp.tile([128, KT * 1024], bf16, tag="w_sb")

                HTG = 2          # chunks per staged hidden group
                ht_tiles = {}

                hT_v3 = hT_b.ap().rearrange("(kt p) t -> p kt t", p=128)
                HTW = HTG * CHUNK

                def emit_ht(g):
                    t0i = g * HTW
                    ht = workp.tile([128, KT * HTW], bf16, tag="ht_blk",
                                    name=f"ht_blk_{g}")
                    htv = ht[:].rearrange("p (kt t) -> p kt t", t=HTW)
                    for k2 in range(0, KT, 2):
                        nc.sync.dma_start(
                            htv[:, k2:k2 + 2, :],
                            hT_v3[:, k2:k2 + 2, t0i:t0i + HTW])
                    ht_tiles[g] = ht

                # interleave weight + first hidden loads so matmul k=0 can
                # start as soon as its two operand tiles have landed
                ht0 = workp.tile([128, KT * HTW], bf16, tag="ht_blk",
                                 name="ht_blk_0")
                ht0v = ht0[:].rearrange("p (kt t) -> p kt t", t=HTW)
                for k in range(KT):
                    nc.sync.dma_start(w_sb[:, k * 1024:(k + 1) * 1024],
                                      w_pack[k * 128:(k + 1) * 128, :])
                    nc.sync.dma_start(ht0v[:, k:k + 1, :],
                                      hT_v3[:, k:k + 1, 0:HTW])
                ht_tiles[0] = ht0
                emit_ht(1)
                aT0 = None
                for tb in range(NTB):
                    if tb == 5:
                        # dense weights load: overlaps phase compute
                        wd_v3 = w_dense_sc.ap().rearrange("(kt p) j -> p kt j", p=128)
                        wdv = wd_sb[:].rearrange("p (kt j) -> p kt j", j=HID)
                        for i4 in range(0, KT, 4):
                            nc.sync.dma_start(wdv[:, i4:i4 + 4, :],
                                              wd_v3[:, i4:i4 + 4, :])
                    t0 = tb * TB * CHUNK
                    v_tb = ringp.tile([128, TB * HPC * D], bf16, tag="v_tb",
                                      name=f"v_tb_{tb}")
                    ks_tb = ringp.tile([128, TB * HPC * D], bf16, tag="ks_tb",
                                       name=f"ks_tb_{tb}")
                    gate_tb = ringp.tile([128, TB * HPC * D], bf16, tag="gate_tb",
                                         name=f"gate_tb_{tb}")
                    attn_tb = ringp.tile([128, TB * HPC * D], bf16, tag="attn_tb",
                                         name=f"attn_tb_{tb}")

                    # ---------- phase 1: projections + norm + rope ----------
                    qk_tiles = []
                    for j in range(TB):
                        c = tb * TB + j
                        g, jj = c // HTG, c % HTG
                        ht_blk = ht_tiles[g]
                        psA = pap.tile([128, 512], f32, tag="psA")
                        psB = pbp.tile([128, 512], f32, tag="psB")
                        for k in range(KT):
                            ht_v = ht_blk[:, k * HTG * CHUNK + jj * 128:
                                          k * HTG * CHUNK + (jj + 1) * 128]
                            nc.tensor.matmul(psA[:], ht_v, w_sb[:, k * 1024:k * 1024 + 512],
                                             start=(k == 0), stop=(k == KT - 1))
                            nc.tensor.matmul(psB[:], ht_v, w_sb[:, k * 1024 + 512:(k + 1) * 1024],
                                             start=(k == 0), stop=(k == KT - 1))
                        if jj == HTG - 1:
                            ht_tiles.pop(g)
                            if g + 2 <= (T // CHUNK - 1) // HTG:
                                emit_ht(g + 2)

                        # early psum evacuation (frees banks for next tile's matmuls)
                        qk_raw = workp.tile([128, 512], f32, tag="qk_raw")
                        nc.vector.tensor_copy(qk_raw[:], psA[:])
                        cs = j * HPC * D
                        nc.vector.tensor_copy(v_tb[:, cs:cs + HPC * D], psB[:, 0:HPC * D])
                        g_raw = workp.tile([128, 256], f32, tag="g_raw")
                        nc.vector.tensor_copy(g_raw[:], psB[:, HPC * D:2 * HPC * D])

                        # RMS stats on raw q/k (per 128-block: q0 q1 k0 k1)
                        sumsq = workp.tile([128, 4], f32, tag="sumsq")
                        sq_scr = workp.tile([128, 128], bf16, tag="sq_scr")
                        for b in range(4):
                            nc.scalar.activation(sq_scr[:], qk_raw[:, b * 128:(b + 1) * 128],
                                                 AF.Square, bias=zero_sb[:, 0:1],
                                                 accum_out=sumsq[:, b:b + 1])
                        srt = workp.tile([128, 4], f32, tag="srt")
                        nc.scalar.activation(srt[:], sumsq[:], AF.Sqrt,
                                             scale=1.0 / D, bias=eps_sb[:, 0:1])
                        rstd = workp.tile([128, 4], f32, tag="rstd")
                        nc.vector.reciprocal(rstd[:], srt[:])

                        # sigmoid gate (from sbuf copy)
                        nc.scalar.activation(gate_tb[:, cs:cs + HPC * D],
                                             g_raw[:], AF.Sigmoid, bias=zero_sb[:, 0:1])

                        # rope on raw values
                        cs_t = workp.tile([128, 256], f32, tag="cs_t")
                        nc.sync.dma_start(cs_t[:], cossin[c * 128:(c + 1) * 128, :])
                        cosv = cs_t[:, 0:128].rearrange("p (b x) -> p b x", x=32)
                        sinv = cs_t[:, 128:256].rearrange("p (b x) -> p b x", x=32)
                        pav = qk_raw[:].rearrange("p (b x) -> p b x", x=128)
                        x1, x2 = pav[:, :, 0:32], pav[:, :, 32:64]
                        rp = workp.tile([128, 512], f32, tag="rp")
                        rpv = rp[:].rearrange("p (b x) -> p b x", x=128)
                        mA = workp.tile([128, 128], f32, tag="mA")
                        mB = workp.tile([128, 128], f32, tag="mB")
                        mAv = mA[:].rearrange("p (b x) -> p b x", x=32)
                        mBv = mB[:].rearrange("p (b x) -> p b x", x=32)
                        nc.vector.tensor_mul(mAv, x1, cosv)
                        nc.vector.tensor_mul(mBv, x2, sinv)
                        nc.vector.tensor_sub(rpv[:, :, 0:32], mAv, mBv)
                        mC = workp.tile([128, 128], f32, tag="mA", name=f"mC_{c}")
                        mD = workp.tile([128, 128], f32, tag="mB", name=f"mD_{c}")
                        mCv = mC[:].rearrange("p (b x) -> p b x", x=32)
                        mDv = mD[:].rearrange("p (b x) -> p b x", x=32)
                        nc.vector.tensor_mul(mCv, x2, cosv)
                        nc.vector.tensor_mul(mDv, x1, sinv)
                        nc.vector.tensor_add(rpv[:, :, 32:64], mCv, mDv)
                        nc.vector.tensor_copy(rpv[:, :, 64:128], pav[:, :, 64:128])
                        if apply_norm_w:
                            nc.vector.tensor_mul(rp[:], rp[:], normw_sb[:])

                        # scale by rstd, cast bf16 (DVE per-partition scalar)
                        qk_nat = workp.tile([128, 512], bf16, tag="qk_nat")
                        for b in range(4):
                            nc.vector.tensor_scalar_mul(
                                qk_nat[:, b * 128:(b + 1) * 128],
                                rp[:, b * 128:(b + 1) * 128], rstd[:, b:b + 1])

                        # k_scaled
                        for h in range(HPC):
                            nc.vector.tensor_scalar_mul(
                                ks_tb[:, cs + h * D:cs + (h + 1) * D],
                                qk_nat[:, 256 + h * 128:256 + (h + 1) * 128],
                                kdec_sb[:, h:h + 1])

                        # transpose q|k on the PE (SBUF->PSUM->SBUF, no DRAM trip)
                        psT = pstp.tile([128, 512], bf16, tag="psT")
                        for b in range(4):
                            nc.tensor.transpose(psT[:, b * 128:(b + 1) * 128],
                                                qk_nat[:, b * 128:(b + 1) * 128],
                                                ident_sb[:])
                        qkT = workp.tile([128, 512], bf16, tag="qkT", bufs=5,
                                         name=f"qkT_{c}")
                        nc.vector.tensor_copy(qkT[:], psT[:])
                        qk_tiles.append(qkT)

                    # ---------- phase 2: GLA scan ----------
                    for j in range(TB):
                        c = tb * TB + j
                        cs = j * HPC * D
                        sk_ps = pskp.tile([128, 512], f32, tag="sk")
                        st_ps = sk_ps[:, 0:256]
                        kv_ps = sk_ps[:, 256:512]
                        o_ps = pop.tile([128, HPC * D], f32, tag="o")
                        qkT = qk_tiles[j]
                        for h in range(HPC):
                            hh = h * 128
                            qTv = qkT[:, h * 128:(h + 1) * 128]
                            kTv = qkT[:, 256 + h * 128:256 + (h + 1) * 128]
                            vv = v_tb[:, cs + h * D:cs + (h + 1) * D]
                            ksv = ks_tb[:, cs + h * D:cs + (h + 1) * D]
                            # St[s,c] = k @ q^T
                            nc.tensor.matmul(st_ps[:, hh:hh + 128], kTv, qTv,
                                             start=True, stop=True)
                            At = workp.tile([128, 128], bf16, tag=f"At{h}",
                                            name=f"At{h}_{c}")
                            nc.vector.tensor_mul(At[:], st_ps[:, hh:hh + 128],
                                                 adect_sb[:, hh:hh + 128])
                            qs = workp.tile([128, 128], bf16, tag=f"qs{h}",
                                            name=f"qs{h}_{c}")
                            nc.vector.tensor_mul(qs[:], qTv, qdecb_sb[:, hh:hh + 128])
                            # o = At.T @ v + qs.T @ S
                            nc.tensor.matmul(o_ps[:, hh:hh + 128], At[:], vv,
                                             start=True, stop=False)
                            nc.tensor.matmul(o_ps[:, hh:hh + 128], qs[:], S_b[:, hh:hh + 128],
                                             start=False, stop=True)
                            # KV = ks.T @ v ; S = sdec*S + KV
                            nc.tensor.matmul(kv_ps[:, hh:hh + 128], ksv, vv,
                                             start=True, stop=True)
                            nc.vector.scalar_tensor_tensor(
                                S_sb[:, hh:hh + 128], S_sb[:, hh:hh + 128],
                                sdec_sb[:, h:h + 1], kv_ps[:, hh:hh + 128],
                                op0=ALU.mult, op1=ALU.add)
                            nc.vector.tensor_copy(S_b[:, hh:hh + 128], S_sb[:, hh:hh + 128])

                        # group rmsnorm (group == this core's 256 dims) + gate
                        gn_scr = workp.tile([128, HPC * D], bf16, tag="gn_scr")
                        gsumsq = workp.tile([128, 1], f32, tag="gsumsq")
                        nc.scalar.activation(gn_scr[:], o_ps[:], AF.Square,
                                             bias=zero_sb[:, 0:1], accum_out=gsumsq[:])
                        gsrt = workp.tile([128, 1], f32, tag="gsrt")
                        nc.scalar.activation(gsrt[:], gsumsq[:], AF.Sqrt,
                                             scale=1.0 / (HPC * D), bias=eps_sb[:, 0:1])
                        grstd = workp.tile([128, 1], f32, tag="grstd")
                        nc.vector.reciprocal(grstd[:], gsrt[:])
                        nc.vector.scalar_tensor_tensor(
                            attn_tb[:, cs:cs + HPC * D], o_ps[:], grstd[:],
                            gate_tb[:, cs:cs + HPC * D],
                            op0=ALU.mult, op1=ALU.mult)

                    # one batched attention write per t-block (3D dst AP)
                    c0_, c1_ = tb * TB, tb * TB + TB
                    seg = 0 if c1_ <= 32 else (1 if c1_ <= 48 else 2)
                    ch0 = c0_ - SEG_CH[seg][0]
                    seg_v3 = attn_seg[seg].ap().rearrange(
                        "(cc p) i -> p cc i", p=128)
                    attn_dma = nc.sync.dma_start(
                        seg_v3[:, ch0:ch0 + TB, :],
                        attn_tb[:].rearrange("p (cc i) -> p cc i", i=HPC * D))
                    if tb in (12, 14):
                        aT_anchor[0 if tb == 12 else 1] = attn_dma

                    # segment collectives fire as soon as their chunks are done
                    if tb == 7:
                        nc.gpsimd.collective_compute(
                            "AllToAll", ALU.bypass,
                            replica_groups=[list(range(N_CORES))],
                            ins=[attn_seg[0].ap().opt()],
                            outs=[att_x[0].ap().opt()],
                        )
                    if tb == 11:
                        nc.gpsimd.collective_compute(
                            "AllToAll", ALU.bypass,
                            replica_groups=[list(range(N_CORES))],
                            ins=[attn_seg[1].ap().opt()],
                            outs=[att_x[1].ap().opt()],
                        )
                    # dense lhsT prep issued mid-loop (after their collective
                    # is truly done, via anchors) so dense matmuls are ready
                    # the moment the phase loop drains
                    if tb == 13:
                        nat0 = emit_nat(0, anchor=aT_anchor[0])
                        aT0 = emit_aTpe(0, nat0)
                    if tb == 15:
                        nat1 = emit_nat(1, anchor=aT_anchor[1])
                        aT1 = emit_aTpe(1, nat1)

                # epilogue: final state out
                for h in range(HPC):
                    nc.sync.dma_start(state_out[h * D:(h + 1) * D, :],
                                      S_sb[:, h * D:(h + 1) * D])

            # last all-to-all (small: 2048 rows)
            nc.gpsimd.collective_compute(
                "AllToAll", ALU.bypass,
                replica_groups=[list(range(N_CORES))],
                ins=[attn_seg[2].ap().opt()],
                outs=[att_x[2].ap().opt()],
            )
            with tc.tile_pool(name="psDp", bufs=2, space="PSUM") as pdp:
                nat2 = emit_nat(2)
                emit_dense(0, aT0, pdp)
                aT2 = emit_aTpe(2, nat2)
                emit_dense(1, aT1, pdp)
                emit_dense(2, aT2, pdp)

    nc.compile()
    return nc


_NC_CACHE: dict = {}


def _get_nc(apply_norm_w: bool):
    key = apply_norm_w
    if key not in _NC_CACHE:
        _NC_CACHE[key] = _build_nc(apply_norm_w)
    return _NC_CACHE[key]


def _host_constants(positions):
    gam = _slopes()  # [H] float64
    c = np.arange(CHUNK, dtype=np.float64)
    dscale = D ** -0.5
    dt_ = c[None, :] - c[:, None]  # [s, c]
    adect = (np.where(dt_ >= 0, np.exp(gam[:, None, None] * dt_[None]), 0.0) * dscale)
    qdec = np.exp(gam[:, None] * (c + 1.0)[None, :]) * dscale        # [H, c]
    kdec = np.exp(gam[:, None] * (CHUNK - 1.0 - c)[None, :])         # [H, s]
    sdec = np.exp(gam * CHUNK)                                       # [H]
    inv = 1.0 / (THETA ** (np.arange(0, RD, 2, dtype=np.float64) / RD))
    ang = positions.astype(np.float64)[:, None] * inv[None, :]       # [T, 32]
    cos = np.cos(ang).astype(np.float32)
    sin = np.sin(ang).astype(np.float32)
    cos4 = np.tile(cos, (1, 4)).astype(np.float32)                   # [T, 128]
    sin4 = np.tile(sin, (1, 4)).astype(np.float32)
    cossin = np.ascontiguousarray(np.concatenate([cos4, sin4], axis=1))
    return (adect.astype(np.float32), qdec.astype(np.float32),
            kdec.astype(np.float32), sdec.astype(np.float32), cossin)


def kernel(positions, hidden_states, recurrent_state, w_qkv, w_g, w_dense,
           q_norm_w, k_norm_w, g_norm_w):
    positions = np.asarray(positions)
    hidden_states = np.asarray(hidden_states, dtype=np.float32)
    recurrent_state = np.asarray(recurrent_state, dtype=np.float32)
    w_qkv = np.asarray(w_qkv, dtype=np.float32)
    w_g = np.asarray(w_g, dtype=np.float32)
    w_dense = np.asarray(w_dense, dtype=np.float32)
    q_norm_w = np.asarray(q_norm_w, dtype=np.float32)
    k_norm_w = np.asarray(k_norm_w, dtype=np.float32)
    g_norm_w = np.asarray(g_norm_w, dtype=np.float32)

    apply_norm_w = not (np.all(q_norm_w == 1.0) and np.all(k_norm_w == 1.0))
    nc = _get_nc(apply_norm_w)

    adect, qdec, kdec, sdec, cossin = _host_constants(positions)
    hT_b = np.ascontiguousarray(hidden_states.T).astype(BF16_NP)
    w_dense_sc = (w_dense * g_norm_w[:, None]).astype(BF16_NP)

    in_maps = []
    for core in range(N_CORES):
        heads = [core * HPC + i for i in range(HPC)]
        cols = []
        for h in heads:
            cols.append(w_qkv[:, h * D:(h + 1) * D])
        for h in heads:
            cols.append(w_qkv[:, H * D + h * D:H * D + (h + 1) * D])
        for h in heads:
            cols.append(w_qkv[:, 2 * H * D + h * D:2 * H * D + (h + 1) * D])
        cols.append(w_g[:, core * HPC * D:(core + 1) * HPC * D])
        w_pack = np.ascontiguousarray(np.concatenate(cols, axis=1)).astype(BF16_NP)

        adect_c = np.ascontiguousarray(
            adect[heads].reshape(HPC * CHUNK, CHUNK))
        # qdecb: [HPC*D, CHUNK], broadcast of qdec over d
        qdecb_c = np.ascontiguousarray(
            np.broadcast_to(qdec[heads][:, None, :], (HPC, D, CHUNK))
            .reshape(HPC * D, CHUNK))
        kdec_c = np.ascontiguousarray(kdec[heads].T)                  # [CHUNK, HPC]
        sdec_c = np.ascontiguousarray(
            np.broadcast_to(sdec[heads][None, :], (CHUNK, HPC))).astype(np.float32)
        state_c = np.ascontiguousarray(
            recurrent_state[heads].reshape(HPC * D, D))

        m = {
            "hT_b": hT_b,
            "w_pack": w_pack,
            "w_dense_sc": w_dense_sc,
            "cossin": cossin,
            "adect": adect_c,
            "qdecb": qdecb_c,
            "kdec": kdec_c,
            "sdec": sdec_c,
            "state_in": state_c,
            "ident": np.eye(128, dtype=np.float32).astype(BF16_NP),
        }
        if apply_norm_w:
            nw = np.concatenate([q_norm_w, q_norm_w, k_norm_w, k_norm_w])
            m["normw"] = np.ascontiguousarray(
                np.broadcast_to(nw[None, :], (CHUNK, 512))).astype(np.float32)
        in_maps.append(m)

    global _last_in_maps
    _last_in_maps = in_maps
    res = bass_utils.run_bass_kernel_spmd(nc, in_maps, core_ids=list(range(N_CORES)))
    results = res.results

    # out_slice[c] = [t in [512c,512c+512) ; [4096+256c,+256) ; [6144+256c,+256)]
    out = np.empty((T, HID), dtype=np.float32)
    for c in range(N_CORES):
        sl = results[c]["out_slice"]
        out[512 * c:512 * (c + 1)] = sl[0:512]
        out[4096 + 256 * c:4096 + 256 * (c + 1)] = sl[512:768]
        out[6144 + 256 * c:6144 + 256 * (c + 1)] = sl[768:1024]
    new_state = np.concatenate(
        [results[c]["state_out"] for c in range(N_CORES)], axis=0
    ).reshape(H, D, D).astype(np.float32)
    return out, new_state


# revision 38
# speedup vs baseline: 1.1062x; 1.0139x over previous
"""Trainium2 Bass kernel for BailingMoeV2.5 linear attention (simple GLA).

Sharding: tensor-parallel over heads. 8 cores x 2 heads each.
  - qkv + gate projections: per-core output-column shards, transposed-hidden
    (precomputed on host, bf16) as the stationary matmul operand.
  - q/k RMSNorm + partial RoPE fused into the projection epilogue.
  - chunked simple-GLA scan (chunk=128), sequential over 64 chunks,
    embarrassingly parallel over heads; fp32 state, bf16 matmuls.
  - GroupRMSNorm group == the core's own 2 heads (local); sigmoid gate.
  - AllGather of bf16 attention output (4.2MB/core), then each core computes
    a 256-column slice of the dense projection (g_norm_w folded into w_dense).
All matmuls bf16 with fp32 PSUM accumulation.
"""
import math
import numpy as np
import ml_dtypes

import concourse.bass as bass
import concourse.bacc as bacc
import concourse.mybir as mybir
import concourse.tile as tile
import concourse.bass_utils as bass_utils

BF16_NP = ml_dtypes.bfloat16
DT = mybir.dt
AF = mybir.ActivationFunctionType
ALU = mybir.AluOpType

T, H, D, HID = 8192, 16, 128, 2048
RD = 64
THETA = 10000.0
EPS = 1e-6
LAYER_IDX, N_LAYERS = 12, 32
CHUNK = 128
NCH = T // CHUNK            # 64 chunks
N_CORES = 8
HPC = H // N_CORES          # 2 heads per core
JS = HID // N_CORES         # 256 output cols per core in dense
TB = 4                      # chunks per t-block (512 t per block)
NTB = NCH // TB             # 16 t-blocks
KT = HID // 128             # 16 k-tiles


def _slopes():
    start = 2.0 ** (-(2.0 ** (-(math.log2(H) - 3))))
    base = np.array([start * start ** i for i in range(H)], dtype=np.float64)
    return base * (-(1.0 - LAYER_IDX / (N_LAYERS - 1) + 1e-5))


def _build_nc(apply_norm_w: bool):
    nc = bacc.Bacc("TRN2", target_bir_lowering=False, debug=False,
                   enable_asserts=False, num_devices=N_CORES)

    f32, bf16 = DT.float32, DT.bfloat16

    # ---- I/O ----
    hT_b = nc.dram_tensor("hT_b", [HID, T], bf16, kind="ExternalInput")
    w_pack = nc.dram_tensor("w_pack", [HID, 4 * HPC * D], bf16, kind="ExternalInput")
    w_dense_sc = nc.dram_tensor("w_dense_sc", [HID, HID], bf16, kind="ExternalInput")
    cossin = nc.dram_tensor("cossin", [T, 256], f32, kind="ExternalInput")
    adect_in = nc.dram_tensor("adect", [HPC * CHUNK, CHUNK], f32, kind="ExternalInput")
    qdecb_in = nc.dram_tensor("qdecb", [HPC * D, CHUNK], f32, kind="ExternalInput")
    kdec_in = nc.dram_tensor("kdec", [CHUNK, HPC], f32, kind="ExternalInput")
    sdec_in = nc.dram_tensor("sdec", [CHUNK, HPC], f32, kind="ExternalInput")
    state_in = nc.dram_tensor("state_in", [HPC * D, D], f32, kind="ExternalInput")
    ident_in = nc.dram_tensor("ident", [128, 128], bf16, kind="ExternalInput")
    if apply_norm_w:
        normw_in = nc.dram_tensor("normw", [CHUNK, 512], f32, kind="ExternalInput")

    TS = T // N_CORES  # 1024: t-slice per core after all-to-all
    out_slice = nc.dram_tensor("out_slice", [TS, HID], f32, kind="ExternalOutput")
    state_out = nc.dram_tensor("state_out", [HPC * D, D], f32, kind="ExternalOutput")

    # ---- internal DRAM ----
    # three t-segments, one all-to-all each (separate tensors so collective
    # deps don't serialize the segments): chunks [0,32), [32,48), [48,64)
    SEG_CH = [(0, 32), (32, 48), (48, 64)]     # chunk ranges
    SEG_ROWS = [(hi - lo) * CHUNK for lo, hi in SEG_CH]
    SEG_SHARD = [r // N_CORES for r in SEG_ROWS]  # per-rank rows: 512/256/256
    attn_seg = [nc.dram_tensor(f"attn_loc{i}", [SEG_ROWS[i], HPC * D], bf16)
                for i in range(3)]
    att_x = [nc.dram_tensor(f"att_x{i}", [SEG_ROWS[i], HPC * D], bf16)
             for i in range(3)]

    with tile.TileContext(nc) as tc:
        with (
            tc.tile_pool(name="const", bufs=1) as cpool,
            tc.tile_pool(name="densew", bufs=1) as dwp,
            tc.tile_pool(name="densework", bufs=1) as dwork,
            tc.tile_pool(name="psT", bufs=1, space="PSUM") as pstp,
        ):
            S_sb = cpool.tile([128, HPC * D], f32, tag="S_sb")
            S_b = cpool.tile([128, HPC * D], bf16, tag="S_b")
            adect_sb = cpool.tile([128, HPC * CHUNK], f32, tag="adect")
            qdecb_sb = cpool.tile([128, HPC * CHUNK], f32, tag="qdecb")
            kdec_sb = cpool.tile([128, HPC], f32, tag="kdec")
            sdec_sb = cpool.tile([128, HPC], f32, tag="sdec")
            if apply_norm_w:
                normw_sb = cpool.tile([128, 512], f32, tag="normw")
                nc.sync.dma_start(normw_sb[:], normw_in[:, :])
            ident_sb = cpool.tile([128, 128], bf16, tag="ident")
            nc.sync.dma_start(ident_sb[:], ident_in[:, :])
            eps_sb = cpool.tile([128, 1], f32, tag="eps")
            zero_sb = cpool.tile([128, 1], f32, tag="zero")
            nc.vector.memset(eps_sb[:], EPS)
            nc.vector.memset(zero_sb[:], 0.0)

            for h in range(HPC):
                nc.sync.dma_start(S_sb[:, h * D:(h + 1) * D],
                                  state_in[h * D:(h + 1) * D, :])
                nc.sync.dma_start(adect_sb[:, h * CHUNK:(h + 1) * CHUNK],
                                  adect_in[h * CHUNK:(h + 1) * CHUNK, :])
                nc.sync.dma_start(qdecb_sb[:, h * CHUNK:(h + 1) * CHUNK],
                                  qdecb_in[h * D:(h + 1) * D, :])
            nc.sync.dma_start(kdec_sb[:], kdec_in[:, :])
            nc.sync.dma_start(sdec_sb[:], sdec_in[:, :])
            nc.vector.tensor_copy(S_b[:], S_sb[:])

            # dense weights: tile at top-level scope, loads emitted later
            wd_sb = dwp.tile([128, KT * HID], bf16, tag="wd_sb")

            aT_anchor = [None, None, None]

            def emit_nat(seg, anchor=None):
                # plain (cheap, non-xbar) loads of the all-to-all result in
                # natural [t, i] layout; anchored so the scheduler doesn't
                # hoist them ahead of the collective (head-of-line blocking)
                rows = SEG_SHARD[seg]
                ntt = rows // 128
                nat = dwork.tile([128, 32 * 256], bf16, tag="nat",
                                 bufs=1, name=f"nat_{seg}")
                natv = nat[:].rearrange("p (b i) -> p b i", i=256)
                xv = att_x[seg].ap().rearrange("(j8 r p) i -> p j8 r i",
                                               p=128, r=ntt)
                for tt in range(ntt):
                    dm = nc.sync.dma_start(
                        natv[:, tt * 8:(tt + 1) * 8, :],
                        xv[:, :, tt, :])
                    if anchor is not None:
                        tile.add_dep_helper(dm.ins, anchor.ins,
                                            reason="nat after anchor")
                return nat

            def emit_aTpe(seg, nat):
                # PE-transpose the natural tiles into [i, t] lhsT layout
                rows = SEG_SHARD[seg]
                ntt = rows // 128
                aT = dwork.tile([128, KT * rows], bf16, tag=f"aT{seg}", bufs=1,
                                name=f"aT_{seg}")
                aTv = aT[:].rearrange("p (i r) -> p i r", r=rows)
                for tt in range(ntt):
                    for r in range(4):
                        psTt = pstp.tile([128, 512], bf16, tag="psT",
                                         name=f"psTd_{seg}_{tt}_{r}")
                        for q in range(4):
                            idx = r * 4 + q
                            nc.tensor.transpose(
                                psTt[:, q * 128:(q + 1) * 128],
                                nat[:, (tt * 8 + idx // 2) * 256 + (idx % 2) * 128:
                                    (tt * 8 + idx // 2) * 256 + (idx % 2 + 1) * 128],
                                ident_sb[:])
                        nc.vector.tensor_copy(
                            aTv[:, 4 * r:4 * (r + 1), tt * 128:(tt + 1) * 128],
                            psTt[:].rearrange("p (i r) -> p i r", r=128))
                return aT

            def emit_dense(seg, aT, pdp):
                rows = SEG_SHARD[seg]
                rbase = sum(SEG_SHARD[:seg])
                for tt in range(rows // 128):
                    for jb in range(4):
                        psD = pdp.tile([128, 512], f32, tag="psD",
                                       name=f"psD_{seg}_{tt}_{jb}")
                        for i in range(KT):
                            lhs = aT[:, i * rows + tt * 128: i * rows + (tt + 1) * 128]
                            nc.tensor.matmul(
                                psD[:], lhs,
                                wd_sb[:, i * HID + jb * 512: i * HID + (jb + 1) * 512],
                                start=(i == 0), stop=(i == KT - 1))
                        oc = dwork.tile([128, 512], f32, tag="oc", bufs=1,
                                        name=f"oc_{seg}_{tt}_{jb}")
                        nc.vector.tensor_copy(oc[:], psD[:])
                        r = rbase + tt * 128
                        nc.sync.dma_start(
                            out_slice[r:r + 128, jb * 512:(jb + 1) * 512], oc[:])

            with (
                tc.tile_pool(name="big", bufs=1) as bigp,
                tc.tile_pool(name="ring", bufs=2) as ringp,
                tc.tile_pool(name="work", bufs=2) as workp,
                tc.tile_pool(name="psA", bufs=2, space="PSUM") as pap,
                tc.tile_pool(name="psB", bufs=2, space="PSUM") as pbp,
                tc.tile_pool(name="psO", bufs=2, space="PSUM") as pop,
                tc.tile_pool(name="psSK", bufs=1, space="PSUM") as pskp,
            ):
                w_sb = bigp.tile([128, KT * 1024], bf16, tag="w_sb")

                HTG = 2          # chunks per staged hidden group
                ht_tiles = {}

                hT_v3 = hT_b.ap().rearrange("(kt p) t -> p kt t", p=128)
                HTW = HTG * CHUNK

                def emit_ht(g):
                    t0i = g * HTW
                    ht = workp.tile([128, KT * HTW], bf16, tag="ht_blk",
                                    name=f"ht_blk_{g}")
                    htv = ht[:].rearrange("p (kt t) -> p kt t", t=HTW)
                    for k2 in range(0, KT, 2):
                        nc.sync.dma_start(
                            htv[:, k2:k2 + 2, :],
                            hT_v3[:, k2:k2 + 2, t0i:t0i + HTW])
                    ht_tiles[g] = ht

                # interleave weight + first hidden loads so matmul k=0 can
                # start as soon as its two operand tiles have landed
                ht0 = workp.tile([128, KT * HTW], bf16, tag="ht_blk",
                                 name="ht_blk_0")
                ht0v = ht0[:].rearrange("p (kt t) -> p kt t", t=HTW)
                for k in range(KT):
                    nc.sync.dma_start(w_sb[:, k * 1024:(k + 1) * 1024],
                                      w_pack[k * 128:(k + 1) * 128, :])
                    nc.sync.dma_start(ht0v[:, k:k + 1, :],
                                      hT_v3[:, k:k + 1, 0:HTW])
                ht_tiles[0] = ht0
                emit_ht(1)
                aT0 = None
                for tb in range(NTB):
                    if tb == 5:
                        # dense weights load: overlaps phase compute
                        wd_v3 = w_dense_sc.ap().rearrange("(kt p) j -> p kt j", p=128)
                        wdv = wd_sb[:].rearrange("p (kt j) -> p kt j", j=HID)
                        for i4 in range(0, KT, 4):
                            nc.sync.dma_start(wdv[:, i4:i4 + 4, :],
                                              wd_v3[:, i4:i4 + 4, :])
                    t0 = tb * TB * CHUNK
                    v_tb = ringp.tile([128, TB * HPC * D], bf16, tag="v_tb",
                                      name=f"v_tb_{tb}")
                    ks_tb = ringp.tile([128, TB * HPC * D], bf16, tag="ks_tb",
                                       name=f"ks_tb_{tb}")
                    gate_tb = ringp.tile([128, TB * HPC * D], bf16, tag="gate_tb",
                                         name=f"gate_tb_{tb}")
                    attn_tb = ringp.tile([128, TB * HPC * D], bf16, tag="attn_tb",
                                         name=f"attn_tb_{tb}")

                    # ---------- phase 1: projections + norm + rope ----------
                    qk_tiles = []
                    for j in range(TB):
                        c = tb * TB + j
                        g, jj = c // HTG, c % HTG
                        ht_blk = ht_tiles[g]
                        psA = pap.tile([128, 512], f32, tag="psA")
                        psB = pbp.tile([128, 512], f32, tag="psB")
                        for k in range(KT):
                            ht_v = ht_blk[:, k * HTG * CHUNK + jj * 128:
                                          k * HTG * CHUNK + (jj + 1) * 128]
                            nc.tensor.matmul(psA[:], ht_v, w_sb[:, k * 1024:k * 1024 + 512],
                                             start=(k == 0), stop=(k == KT - 1))
                            nc.tensor.matmul(psB[:], ht_v, w_sb[:, k * 1024 + 512:(k + 1) * 1024],
                                             start=(k == 0), stop=(k == KT - 1))
                        if jj == HTG - 1:
                            ht_tiles.pop(g)
                            if g + 2 <= (T // CHUNK - 1) // HTG:
                                emit_ht(g + 2)

                        # early psum evacuation (frees banks for next tile's matmuls)
                        qk_raw = workp.tile([128, 512], f32, tag="qk_raw")
                        nc.vector.tensor_copy(qk_raw[:], psA[:])
                        cs = j * HPC * D
                        nc.vector.tensor_copy(v_tb[:, cs:cs + HPC * D], psB[:, 0:HPC * D])
                        g_raw = workp.tile([128, 256], f32, tag="g_raw")
                        nc.vector.tensor_copy(g_raw[:], psB[:, HPC * D:2 * HPC * D])

                        # RMS stats on raw q/k (per 128-block: q0 q1 k0 k1)
                        sumsq = workp.tile([128, 4], f32, tag="sumsq")
                        sq_scr = workp.tile([128, 128], bf16, tag="sq_scr")
                        for b in range(4):
                            nc.scalar.activation(sq_scr[:], qk_raw[:, b * 128:(b + 1) * 128],
                                                 AF.Square, bias=zero_sb[:, 0:1],
                                                 accum_out=sumsq[:, b:b + 1])
                        srt = workp.tile([128, 4], f32, tag="srt")
                        nc.scalar.activation(srt[:], sumsq[:], AF.Sqrt,
                                             scale=1.0 / D, bias=eps_sb[:, 0:1])
                        rstd = workp.tile([128, 4], f32, tag="rstd")
                        nc.vector.reciprocal(rstd[:], srt[:])

                        # sigmoid gate (from sbuf copy)
                        nc.scalar.activation(gate_tb[:, cs:cs + HPC * D],
                                             g_raw[:], AF.Sigmoid, bias=zero_sb[:, 0:1])

                        # rope on raw values
                        cs_t = workp.tile([128, 256], f32, tag="cs_t")
                        nc.sync.dma_start(cs_t[:], cossin[c * 128:(c + 1) * 128, :])
                        cosv = cs_t[:, 0:128].rearrange("p (b x) -> p b x", x=32)
                        sinv = cs_t[:, 128:256].rearrange("p (b x) -> p b x", x=32)
                        pav = qk_raw[:].rearrange("p (b x) -> p b x", x=128)
                        x1, x2 = pav[:, :, 0:32], pav[:, :, 32:64]
                        rp = workp.tile([128, 512], f32, tag="rp")
                        rpv = rp[:].rearrange("p (b x) -> p b x", x=128)
                        mA = workp.tile([128, 128], f32, tag="mA")
                        mB = workp.tile([128, 128], f32, tag="mB")
                        mAv = mA[:].rearrange("p (b x) -> p b x", x=32)
                        mBv = mB[:].rearrange("p (b x) -> p b x", x=32)
                        nc.vector.tensor_mul(mAv, x1, cosv)
                        nc.vector.tensor_mul(mBv, x2, sinv)
                        nc.vector.tensor_sub(rpv[:, :, 0:32], mAv, mBv)
                        mC = workp.tile([128, 128], f32, tag="mA", name=f"mC_{c}")
                        mD = workp.tile([128, 128], f32, tag="mB", name=f"mD_{c}")
                        mCv = mC[:].rearrange("p (b x) -> p b x", x=32)
                        mDv = mD[:].rearrange("p (b x) -> p b x", x=32)
                        nc.vector.tensor_mul(mCv, x2, cosv)
                        nc.vector.tensor_mul(mDv, x1, sinv)
                        nc.vector.tensor_add(rpv[:, :, 32:64], mCv, mDv)
                        nc.vector.tensor_copy(rpv[:, :, 64:128], pav[:, :, 64:128])
                        if apply_norm_w:
                            nc.vector.tensor_mul(rp[:], rp[:], normw_sb[:])

                        # scale by rstd, cast bf16 (DVE per-partition scalar)
                        qk_nat = workp.tile([128, 512], bf16, tag="qk_nat")
                        for b in range(4):
                            nc.vector.tensor_scalar_mul(
                                qk_nat[:, b * 128:(b + 1) * 128],
                                rp[:, b * 128:(b + 1) * 128], rstd[:, b:b + 1])

                        # k_scaled
                        for h in range(HPC):
                            nc.vector.tensor_scalar_mul(
                                ks_tb[:, cs + h * D:cs + (h + 1) * D],
                                qk_nat[:, 256 + h * 128:256 + (h + 1) * 128],
                                kdec_sb[:, h:h + 1])

                        # transpose q|k on the PE (SBUF->PSUM->SBUF, no DRAM trip)
                        psT = pstp.tile([128, 512], bf16, tag="psT")
                        for b in range(4):
                            nc.tensor.transpose(psT[:, b * 128:(b + 1) * 128],
                                                qk_nat[:, b * 128:(b + 1) * 128],
                                                ident_sb[:])
                        qkT = workp.tile([128, 512], bf16, tag="qkT", bufs=5,
                                         name=f"qkT_{c}")
                        nc.vector.tensor_copy(qkT[:], psT[:])
                        qk_tiles.append(qkT)

                    # ---------- phase 2: GLA scan ----------
                    for j in range(TB):
                        c = tb * TB + j
                        cs = j * HPC * D
                        sk_ps = pskp.tile([128, 512], f32, tag="sk")
                        st_ps = sk_ps[:, 0:256]
                        kv_ps = sk_ps[:, 256:512]
                        o_ps = pop.tile([128, HPC * D], f32, tag="o")
                        qkT = qk_tiles[j]
                        for h in range(HPC):
                            hh = h * 128
                            qTv = qkT[:, h * 128:(h + 1) * 128]
                            kTv = qkT[:, 256 + h * 128:256 + (h + 1) * 128]
                            vv = v_tb[:, cs + h * D:cs + (h + 1) * D]
                            ksv = ks_tb[:, cs + h * D:cs + (h + 1) * D]
                            # St[s,c] = k @ q^T
                            nc.tensor.matmul(st_ps[:, hh:hh + 128], kTv, qTv,
                                             start=True, stop=True)
                            At = workp.tile([128, 128], bf16, tag=f"At{h}",
                                            name=f"At{h}_{c}")
                            nc.vector.tensor_mul(At[:], st_ps[:, hh:hh + 128],
                                                 adect_sb[:, hh:hh + 128])
                            qs = workp.tile([128, 128], bf16, tag=f"qs{h}",
                                            name=f"qs{h}_{c}")
                            nc.vector.tensor_mul(qs[:], qTv, qdecb_sb[:, hh:hh + 128])
                            # o = At.T @ v + qs.T @ S
                            nc.tensor.matmul(o_ps[:, hh:hh + 128], At[:], vv,
                                             start=True, stop=False)
                            nc.tensor.matmul(o_ps[:, hh:hh + 128], qs[:], S_b[:, hh:hh + 128],
                                             start=False, stop=True)
                            # KV = ks.T @ v ; S = sdec*S + KV
                            nc.tensor.matmul(kv_ps[:, hh:hh + 128], ksv, vv,
                                             start=True, stop=True)
                            nc.vector.scalar_tensor_tensor(
                                S_sb[:, hh:hh + 128], S_sb[:, hh:hh + 128],
                                sdec_sb[:, h:h + 1], kv_ps[:, hh:hh + 128],
                                op0=ALU.mult, op1=ALU.add)
                            nc.vector.tensor_copy(S_b[:, hh:hh + 128], S_sb[:, hh:hh + 128])

                        # group rmsnorm (group == this core's 256 dims) + gate
                        gn_scr = workp.tile([128, HPC * D], bf16, tag="gn_scr")
                        gsumsq = workp.tile([128, 1], f32, tag="gsumsq")
                        nc.scalar.activation(gn_scr[:], o_ps[:], AF.Square,
                                             bias=zero_sb[:, 0:1], accum_out=gsumsq[:])
                        gsrt = workp.tile([128, 1], f32, tag="gsrt")
                        nc.scalar.activation(gsrt[:], gsumsq[:], AF.Sqrt,
                                             scale=1.0 / (HPC * D), bias=eps_sb[:, 0:1])
                        grstd = workp.tile([128, 1], f32, tag="grstd")
                        nc.vector.reciprocal(grstd[:], gsrt[:])
                        nc.vector.scalar_tensor_tensor(
                            attn_tb[:, cs:cs + HPC * D], o_ps[:], grstd[:],
                            gate_tb[:, cs:cs + HPC * D],
                            op0=ALU.mult, op1=ALU.mult)

                    # one batched attention write per t-block (3D dst AP)
                    c0_, c1_ = tb * TB, tb * TB + TB
                    seg = 0 if c1_ <= 32 else (1 if c1_ <= 48 else 2)
                    ch0 = c0_ - SEG_CH[seg][0]
                    seg_v3 = attn_seg[seg].ap().rearrange(
                        "(cc p) i -> p cc i", p=128)
                    attn_dma = nc.sync.dma_start(
                        seg_v3[:, ch0:ch0 + TB, :],
                        attn_tb[:].rearrange("p (cc i) -> p cc i", i=HPC * D))
                    if tb in (12, 14):
                        aT_anchor[0 if tb == 12 else 1] = attn_dma

                    # segment collectives fire as soon as their chunks are done
                    if tb == 7:
                        nc.gpsimd.collective_compute(
                            "AllToAll", ALU.bypass,
                            replica_groups=[list(range(N_CORES))],
                            ins=[attn_seg[0].ap().opt()],
                            outs=[att_x[0].ap().opt()],
                        )
                    if tb == 11:
                        nc.gpsimd.collective_compute(
                            "AllToAll", ALU.bypass,
                            replica_groups=[list(range(N_CORES))],
                            ins=[attn_seg[1].ap().opt()],
                            outs=[att_x[1].ap().opt()],
                        )
                    # dense lhsT prep issued mid-loop (after their collective
                    # is truly done, via anchors) so dense matmuls are ready
                    # the moment the phase loop drains
                    if tb == 13:
                        nat0 = emit_nat(0, anchor=aT_anchor[0])
                        aT0 = emit_aTpe(0, nat0)
                    if tb == 15:
                        nat1 = emit_nat(1, anchor=aT_anchor[1])
                        aT1 = emit_aTpe(1, nat1)

                # epilogue: final state out
                for h in range(HPC):
                    nc.sync.dma_start(state_out[h * D:(h + 1) * D, :],
                                      S_sb[:, h * D:(h + 1) * D])

            # last all-to-all (small: 2048 rows)
            nc.gpsimd.collective_compute(
                "AllToAll", ALU.bypass,
                replica_groups=[list(range(N_CORES))],
                ins=[attn_seg[2].ap().opt()],
                outs=[att_x[2].ap().opt()],
            )
            with tc.tile_pool(name="psDp", bufs=2, space="PSUM") as pdp:
                emit_dense(0, aT0, pdp)
                nat2 = emit_nat(2)
                aT2 = emit_aTpe(2, nat2)
                emit_dense(1, aT1, pdp)
                emit_dense(2, aT2, pdp)

    nc.compile()
    return nc


_NC_CACHE: dict = {}


def _get_nc(apply_norm_w: bool):
    key = apply_norm_w
    if key not in _NC_CACHE:
        _NC_CACHE[key] = _build_nc(apply_norm_w)
    return _NC_CACHE[key]


def _host_constants(positions):
    gam = _slopes()  # [H] float64
    c = np.arange(CHUNK, dtype=np.float64)
    dscale = D ** -0.5
    dt_ = c[None, :] - c[:, None]  # [s, c]
    adect = (np.where(dt_ >= 0, np.exp(gam[:, None, None] * dt_[None]), 0.0) * dscale)
    qdec = np.exp(gam[:, None] * (c + 1.0)[None, :]) * dscale        # [H, c]
    kdec = np.exp(gam[:, None] * (CHUNK - 1.0 - c)[None, :])         # [H, s]
    sdec = np.exp(gam * CHUNK)                                       # [H]
    inv = 1.0 / (THETA ** (np.arange(0, RD, 2, dtype=np.float64) / RD))
    ang = positions.astype(np.float64)[:, None] * inv[None, :]       # [T, 32]
    cos = np.cos(ang).astype(np.float32)
    sin = np.sin(ang).astype(np.float32)
    cos4 = np.tile(cos, (1, 4)).astype(np.float32)                   # [T, 128]
    sin4 = np.tile(sin, (1, 4)).astype(np.float32)
    cossin = np.ascontiguousarray(np.concatenate([cos4, sin4], axis=1))
    return (adect.astype(np.float32), qdec.astype(np.float32),
            kdec.astype(np.float32), sdec.astype(np.float32), cossin)


def kernel(positions, hidden_states, recurrent_state, w_qkv, w_g, w_dense,
           q_norm_w, k_norm_w, g_norm_w):
    positions = np.asarray(positions)
    hidden_states = np.asarray(hidden_states, dtype=np.float32)
    recurrent_state = np.asarray(recurrent_state, dtype=np.float32)
    w_qkv = np.asarray(w_qkv, dtype=np.float32)
    w_g = np.asarray(w_g, dtype=np.float32)
    w_dense = np.asarray(w_dense, dtype=np.float32)
    q_norm_w = np.asarray(q_norm_w, dtype=np.float32)
    k_norm_w = np.asarray(k_norm_w, dtype=np.float32)
    g_norm_w = np.asarray(g_norm_w, dtype=np.float32)

    apply_norm_w = not (np.all(q_norm_w == 1.0) and np.all(k_norm_w == 1.0))
    nc = _get_nc(apply_norm_w)

    adect, qdec, kdec, sdec, cossin = _host_constants(positions)
    hT_b = np.ascontiguousarray(hidden_states.T).astype(BF16_NP)
    w_dense_sc = (w_dense * g_norm_w[:, None]).astype(BF16_NP)

    in_maps = []
    for core in range(N_CORES):
        heads = [core * HPC + i for i in range(HPC)]
        cols = []
        for h in heads:
            cols.append(w_qkv[:, h * D:(h + 1) * D])
        for h in heads:
            cols.append(w_qkv[:, H * D + h * D:H * D + (h + 1) * D])
        for h in heads:
            cols.append(w_qkv[:, 2 * H * D + h * D:2 * H * D + (h + 1) * D])
        cols.append(w_g[:, core * HPC * D:(core + 1) * HPC * D])
        w_pack = np.ascontiguousarray(np.concatenate(cols, axis=1)).astype(BF16_NP)

        adect_c = np.ascontiguousarray(
            adect[heads].reshape(HPC * CHUNK, CHUNK))
        # qdecb: [HPC*D, CHUNK], broadcast of qdec over d
        qdecb_c = np.ascontiguousarray(
            np.broadcast_to(qdec[heads][:, None, :], (HPC, D, CHUNK))
            .reshape(HPC * D, CHUNK))
        kdec_c = np.ascontiguousarray(kdec[heads].T)                  # [CHUNK, HPC]
        sdec_c = np.ascontiguousarray(
            np.broadcast_to(sdec[heads][None, :], (CHUNK, HPC))).astype(np.float32)
        state_c = np.ascontiguousarray(
            recurrent_state[heads].reshape(HPC * D, D))

        m = {
            "hT_b": hT_b,
            "w_pack": w_pack,
            "w_dense_sc": w_dense_sc,
            "cossin": cossin,
            "adect": adect_c,
            "qdecb": qdecb_c,
            "kdec": kdec_c,
            "sdec": sdec_c,
            "state_in": state_c,
            "ident": np.eye(128, dtype=np.float32).astype(BF16_NP),
        }
        if apply_norm_w:
            nw = np.concatenate([q_norm_w, q_norm_w, k_norm_w, k_norm_w])
            m["normw"] = np.ascontiguousarray(
                np.broadcast_to(nw[None, :], (CHUNK, 512))).astype(np.float32)
        in_maps.append(m)

    global _last_in_maps
    _last_in_maps = in_maps
    res = bass_utils.run_bass_kernel_spmd(nc, in_maps, core_ids=list(range(N_CORES)))
    results = res.results

    # out_slice[c] = [t in [512c,512c+512) ; [4096+256c,+256) ; [6144+256c,+256)]
    out = np.empty((T, HID), dtype=np.float32)
    for c in range(N_CORES):
        sl = results[c]["out_slice"]
        out[512 * c:512 * (c + 1)] = sl[0:512]
        out[4096 + 256 * c:4096 + 256 * (c + 1)] = sl[512:768]
        out[6144 + 256 * c:6144 + 256 * (c + 1)] = sl[768:1024]
    new_state = np.concatenate(
        [results[c]["state_out"] for c in range(N_CORES)], axis=0
    ).reshape(H, D, D).astype(np.float32)
    return out, new_state


# revision 40
# speedup vs baseline: 1.1103x; 1.0037x over previous
"""Trainium2 Bass kernel for BailingMoeV2.5 linear attention (simple GLA).

Sharding: tensor-parallel over heads. 8 cores x 2 heads each.
  - qkv + gate projections: per-core output-column shards, transposed-hidden
    (precomputed on host, bf16) as the stationary matmul operand.
  - q/k RMSNorm + partial RoPE fused into the projection epilogue.
  - chunked simple-GLA scan (chunk=128), sequential over 64 chunks,
    embarrassingly parallel over heads; fp32 state, bf16 matmuls.
  - GroupRMSNorm group == the core's own 2 heads (local); sigmoid gate.
  - bf16 attention output redistributed with three overlapped AllToAll
    collectives (t-segments of 32/16/16 chunks); each core then computes the
    dense projection for its own t-slice against the full w_dense (g_norm_w
    folded into w_dense on the host), so no output reduction is needed.
  - All DMA-transposes replaced by PE-transposes (identity matmuls) to avoid
    the xbar/collective serialization; DMAs batched via 3D access patterns.
All matmuls bf16 with fp32 PSUM accumulation; state/stats/norms in fp32.
"""
import math
import numpy as np
import ml_dtypes

import concourse.bass as bass
import concourse.bacc as bacc
import concourse.mybir as mybir
import concourse.tile as tile
import concourse.bass_utils as bass_utils

BF16_NP = ml_dtypes.bfloat16
DT = mybir.dt
AF = mybir.ActivationFunctionType
ALU = mybir.AluOpType

T, H, D, HID = 8192, 16, 128, 2048
RD = 64
THETA = 10000.0
EPS = 1e-6
LAYER_IDX, N_LAYERS = 12, 32
CHUNK = 128
NCH = T // CHUNK            # 64 chunks
N_CORES = 8
HPC = H // N_CORES          # 2 heads per core
JS = HID // N_CORES         # 256 output cols per core in dense
TB = 4                      # chunks per t-block (512 t per block)
NTB = NCH // TB             # 16 t-blocks
KT = HID // 128             # 16 k-tiles


def _slopes():
    start = 2.0 ** (-(2.0 ** (-(math.log2(H) - 3))))
    base = np.array([start * start ** i for i in range(H)], dtype=np.float64)
    return base * (-(1.0 - LAYER_IDX / (N_LAYERS - 1) + 1e-5))


def _build_nc(apply_norm_w: bool):
    nc = bacc.Bacc("TRN2", target_bir_lowering=False, debug=False,
                   enable_asserts=False, num_devices=N_CORES)

    f32, bf16 = DT.float32, DT.bfloat16

    # ---- I/O ----
    hT_b = nc.dram_tensor("hT_b", [HID, T], bf16, kind="ExternalInput")
    w_pack = nc.dram_tensor("w_pack", [HID, 4 * HPC * D], bf16, kind="ExternalInput")
    w_dense_sc = nc.dram_tensor("w_dense_sc", [HID, HID], bf16, kind="ExternalInput")
    cossin = nc.dram_tensor("cossin", [T, 256], f32, kind="ExternalInput")
    adect_in = nc.dram_tensor("adect", [HPC * CHUNK, CHUNK], f32, kind="ExternalInput")
    qdecb_in = nc.dram_tensor("qdecb", [HPC * D, CHUNK], f32, kind="ExternalInput")
    kdec_in = nc.dram_tensor("kdec", [CHUNK, HPC], f32, kind="ExternalInput")
    sdec_in = nc.dram_tensor("sdec", [CHUNK, HPC], f32, kind="ExternalInput")
    state_in = nc.dram_tensor("state_in", [HPC * D, D], f32, kind="ExternalInput")
    ident_in = nc.dram_tensor("ident", [128, 128], bf16, kind="ExternalInput")
    if apply_norm_w:
        normw_in = nc.dram_tensor("normw", [CHUNK, 512], f32, kind="ExternalInput")

    TS = T // N_CORES  # 1024: t-slice per core after all-to-all
    out_slice = nc.dram_tensor("out_slice", [TS, HID], f32, kind="ExternalOutput")
    state_out = nc.dram_tensor("state_out", [HPC * D, D], f32, kind="ExternalOutput")

    # ---- internal DRAM ----
    # three t-segments, one all-to-all each (separate tensors so collective
    # deps don't serialize the segments): chunks [0,32), [32,48), [48,64)
    SEG_CH = [(0, 32), (32, 48), (48, 64)]     # chunk ranges
    SEG_ROWS = [(hi - lo) * CHUNK for lo, hi in SEG_CH]
    SEG_SHARD = [r // N_CORES for r in SEG_ROWS]  # per-rank rows: 512/256/256
    attn_seg = [nc.dram_tensor(f"attn_loc{i}", [SEG_ROWS[i], HPC * D], bf16)
                for i in range(3)]
    att_x = [nc.dram_tensor(f"att_x{i}", [SEG_ROWS[i], HPC * D], bf16)
             for i in range(3)]

    with tile.TileContext(nc) as tc:
        with (
            tc.tile_pool(name="const", bufs=1) as cpool,
            tc.tile_pool(name="densew", bufs=1) as dwp,
            tc.tile_pool(name="densework", bufs=1) as dwork,
            tc.tile_pool(name="psT", bufs=1, space="PSUM") as pstp,
        ):
            S_sb = cpool.tile([128, HPC * D], f32, tag="S_sb")
            S_b = cpool.tile([128, HPC * D], bf16, tag="S_b")
            adect_sb = cpool.tile([128, HPC * CHUNK], f32, tag="adect")
            qdecb_sb = cpool.tile([128, HPC * CHUNK], f32, tag="qdecb")
            kdec_sb = cpool.tile([128, HPC], f32, tag="kdec")
            sdec_sb = cpool.tile([128, HPC], f32, tag="sdec")
            if apply_norm_w:
                normw_sb = cpool.tile([128, 512], f32, tag="normw")
                nc.sync.dma_start(normw_sb[:], normw_in[:, :])
            ident_sb = cpool.tile([128, 128], bf16, tag="ident")
            nc.sync.dma_start(ident_sb[:], ident_in[:, :])
            eps_sb = cpool.tile([128, 1], f32, tag="eps")
            zero_sb = cpool.tile([128, 1], f32, tag="zero")
            nc.vector.memset(eps_sb[:], EPS)
            nc.vector.memset(zero_sb[:], 0.0)

            for h in range(HPC):
                nc.sync.dma_start(S_sb[:, h * D:(h + 1) * D],
                                  state_in[h * D:(h + 1) * D, :])
                nc.sync.dma_start(adect_sb[:, h * CHUNK:(h + 1) * CHUNK],
                                  adect_in[h * CHUNK:(h + 1) * CHUNK, :])
                nc.sync.dma_start(qdecb_sb[:, h * CHUNK:(h + 1) * CHUNK],
                                  qdecb_in[h * D:(h + 1) * D, :])
            nc.sync.dma_start(kdec_sb[:], kdec_in[:, :])
            nc.sync.dma_start(sdec_sb[:], sdec_in[:, :])
            nc.vector.tensor_copy(S_b[:], S_sb[:])

            # dense weights: tile at top-level scope, loads emitted later
            wd_sb = dwp.tile([128, KT * HID], bf16, tag="wd_sb")

            aT_anchor = [None, None, None]

            def emit_nat(seg, anchor=None):
                # plain (cheap, non-xbar) loads of the all-to-all result in
                # natural [t, i] layout; anchored so the scheduler doesn't
                # hoist them ahead of the collective (head-of-line blocking)
                rows = SEG_SHARD[seg]
                ntt = rows // 128
                nat = dwork.tile([128, 32 * 256], bf16, tag="nat",
                                 bufs=1, name=f"nat_{seg}")
                natv = nat[:].rearrange("p (b i) -> p b i", i=256)
                xv = att_x[seg].ap().rearrange("(j8 r p) i -> p j8 r i",
                                               p=128, r=ntt)
                for tt in range(ntt):
                    dm = nc.sync.dma_start(
                        natv[:, tt * 8:(tt + 1) * 8, :],
                        xv[:, :, tt, :])
                    if anchor is not None:
                        tile.add_dep_helper(dm.ins, anchor.ins,
                                            reason="nat after anchor")
                return nat

            def emit_aTpe(seg, nat):
                # PE-transpose the natural tiles into [i, t] lhsT layout
                rows = SEG_SHARD[seg]
                ntt = rows // 128
                aT = dwork.tile([128, KT * rows], bf16, tag=f"aT{seg}", bufs=1,
                                name=f"aT_{seg}")
                aTv = aT[:].rearrange("p (i r) -> p i r", r=rows)
                for tt in range(ntt):
                    for r in range(4):
                        psTt = pstp.tile([128, 512], bf16, tag="psT",
                                         name=f"psTd_{seg}_{tt}_{r}")
                        for q in range(4):
                            idx = r * 4 + q
                            nc.tensor.transpose(
                                psTt[:, q * 128:(q + 1) * 128],
                                nat[:, (tt * 8 + idx // 2) * 256 + (idx % 2) * 128:
                                    (tt * 8 + idx // 2) * 256 + (idx % 2 + 1) * 128],
                                ident_sb[:])
                        nc.vector.tensor_copy(
                            aTv[:, 4 * r:4 * (r + 1), tt * 128:(tt + 1) * 128],
                            psTt[:].rearrange("p (i r) -> p i r", r=128))
                return aT

            def emit_dense(seg, aT, pdp):
                rows = SEG_SHARD[seg]
                rbase = sum(SEG_SHARD[:seg])
                for tt in range(rows // 128):
                    for jb in range(4):
                        psD = pdp.tile([128, 512], f32, tag="psD",
                                       name=f"psD_{seg}_{tt}_{jb}")
                        for i in range(KT):
                            lhs = aT[:, i * rows + tt * 128: i * rows + (tt + 1) * 128]
                            nc.tensor.matmul(
                                psD[:], lhs,
                                wd_sb[:, i * HID + jb * 512: i * HID + (jb + 1) * 512],
                                start=(i == 0), stop=(i == KT - 1))
                        oc = dwork.tile([128, 512], f32, tag="oc", bufs=1,
                                        name=f"oc_{seg}_{tt}_{jb}")
                        nc.vector.tensor_copy(oc[:], psD[:])
                        r = rbase + tt * 128
                        nc.sync.dma_start(
                            out_slice[r:r + 128, jb * 512:(jb + 1) * 512], oc[:])

            with (
                tc.tile_pool(name="big", bufs=1) as bigp,
                tc.tile_pool(name="ring", bufs=2) as ringp,
                tc.tile_pool(name="work", bufs=2) as workp,
                tc.tile_pool(name="psA", bufs=2, space="PSUM") as pap,
                tc.tile_pool(name="psB", bufs=2, space="PSUM") as pbp,
                tc.tile_pool(name="psO", bufs=2, space="PSUM") as pop,
                tc.tile_pool(name="psSK", bufs=1, space="PSUM") as pskp,
            ):
                w_sb = bigp.tile([128, KT * 1024], bf16, tag="w_sb")

                HTG = 2          # chunks per staged hidden group
                ht_tiles = {}

                hT_v3 = hT_b.ap().rearrange("(kt p) t -> p kt t", p=128)
                HTW = HTG * CHUNK

                def emit_ht(g):
                    t0i = g * HTW
                    ht = workp.tile([128, KT * HTW], bf16, tag="ht_blk",
                                    name=f"ht_blk_{g}")
                    htv = ht[:].rearrange("p (kt t) -> p kt t", t=HTW)
                    for k2 in range(0, KT, 2):
                        nc.sync.dma_start(
                            htv[:, k2:k2 + 2, :],
                            hT_v3[:, k2:k2 + 2, t0i:t0i + HTW])
                    ht_tiles[g] = ht

                # interleave weight + first hidden loads so matmul k=0 can
                # start as soon as its two operand tiles have landed
                ht0 = workp.tile([128, KT * HTW], bf16, tag="ht_blk",
                                 name="ht_blk_0")
                ht0v = ht0[:].rearrange("p (kt t) -> p kt t", t=HTW)
                for k in range(KT):
                    nc.sync.dma_start(w_sb[:, k * 1024:(k + 1) * 1024],
                                      w_pack[k * 128:(k + 1) * 128, :])
                    nc.sync.dma_start(ht0v[:, k:k + 1, :],
                                      hT_v3[:, k:k + 1, 0:HTW])
                ht_tiles[0] = ht0
                emit_ht(1)
                aT0 = None
                for tb in range(NTB):
                    if tb == 5:
                        # dense weights load: overlaps phase compute
                        wd_v3 = w_dense_sc.ap().rearrange("(kt p) j -> p kt j", p=128)
                        wdv = wd_sb[:].rearrange("p (kt j) -> p kt j", j=HID)
                        for i4 in range(0, KT, 4):
                            nc.sync.dma_start(wdv[:, i4:i4 + 4, :],
                                              wd_v3[:, i4:i4 + 4, :])
                    t0 = tb * TB * CHUNK
                    v_tb = ringp.tile([128, TB * HPC * D], bf16, tag="v_tb",
                                      name=f"v_tb_{tb}")
                    ks_tb = ringp.tile([128, TB * HPC * D], bf16, tag="ks_tb",
                                       name=f"ks_tb_{tb}")
                    gate_tb = ringp.tile([128, TB * HPC * D], bf16, tag="gate_tb",
                                         name=f"gate_tb_{tb}")
                    attn_tb = ringp.tile([128, TB * HPC * D], bf16, tag="attn_tb",
                                         name=f"attn_tb_{tb}")

                    # ---------- phase 1: projections + norm + rope ----------
                    qk_tiles = []
                    for j in range(TB):
                        c = tb * TB + j
                        g, jj = c // HTG, c % HTG
                        ht_blk = ht_tiles[g]
                        psA = pap.tile([128, 512], f32, tag="psA")
                        psB = pbp.tile([128, 512], f32, tag="psB")
                        for k in range(KT):
                            ht_v = ht_blk[:, k * HTG * CHUNK + jj * 128:
                                          k * HTG * CHUNK + (jj + 1) * 128]
                            nc.tensor.matmul(psA[:], ht_v, w_sb[:, k * 1024:k * 1024 + 512],
                                             start=(k == 0), stop=(k == KT - 1))
                            nc.tensor.matmul(psB[:], ht_v, w_sb[:, k * 1024 + 512:(k + 1) * 1024],
                                             start=(k == 0), stop=(k == KT - 1))
                        if jj == HTG - 1:
                            ht_tiles.pop(g)
                            if g + 2 <= (T // CHUNK - 1) // HTG:
                                emit_ht(g + 2)

                        # early psum evacuation (frees banks for next tile's matmuls)
                        qk_raw = workp.tile([128, 512], f32, tag="qk_raw")
                        nc.vector.tensor_copy(qk_raw[:], psA[:])
                        cs = j * HPC * D
                        nc.vector.tensor_copy(v_tb[:, cs:cs + HPC * D], psB[:, 0:HPC * D])
                        g_raw = workp.tile([128, 256], f32, tag="g_raw")
                        nc.vector.tensor_copy(g_raw[:], psB[:, HPC * D:2 * HPC * D])

                        # RMS stats on raw q/k (per 128-block: q0 q1 k0 k1)
                        sumsq = workp.tile([128, 4], f32, tag="sumsq")
                        sq_scr = workp.tile([128, 128], bf16, tag="sq_scr")
                        for b in range(4):
                            nc.scalar.activation(sq_scr[:], qk_raw[:, b * 128:(b + 1) * 128],
                                                 AF.Square, bias=zero_sb[:, 0:1],
                                                 accum_out=sumsq[:, b:b + 1])
                        srt = workp.tile([128, 4], f32, tag="srt")
                        nc.scalar.activation(srt[:], sumsq[:], AF.Sqrt,
                                             scale=1.0 / D, bias=eps_sb[:, 0:1])
                        rstd = workp.tile([128, 4], f32, tag="rstd")
                        nc.vector.reciprocal(rstd[:], srt[:])

                        # sigmoid gate (from sbuf copy)
                        nc.scalar.activation(gate_tb[:, cs:cs + HPC * D],
                                             g_raw[:], AF.Sigmoid, bias=zero_sb[:, 0:1])

                        # rope on raw values
                        cs_t = workp.tile([128, 256], f32, tag="cs_t")
                        nc.sync.dma_start(cs_t[:], cossin[c * 128:(c + 1) * 128, :])
                        cosv = cs_t[:, 0:128].rearrange("p (b x) -> p b x", x=32)
                        sinv = cs_t[:, 128:256].rearrange("p (b x) -> p b x", x=32)
                        pav = qk_raw[:].rearrange("p (b x) -> p b x", x=128)
                        x1, x2 = pav[:, :, 0:32], pav[:, :, 32:64]
                        rp = workp.tile([128, 512], f32, tag="rp")
                        rpv = rp[:].rearrange("p (b x) -> p b x", x=128)
                        mA = workp.tile([128, 128], f32, tag="mA")
                        mB = workp.tile([128, 128], f32, tag="mB")
                        mAv = mA[:].rearrange("p (b x) -> p b x", x=32)
                        mBv = mB[:].rearrange("p (b x) -> p b x", x=32)
                        nc.vector.tensor_mul(mAv, x1, cosv)
                        nc.vector.tensor_mul(mBv, x2, sinv)
                        nc.vector.tensor_sub(rpv[:, :, 0:32], mAv, mBv)
                        mC = workp.tile([128, 128], f32, tag="mA", name=f"mC_{c}")
                        mD = workp.tile([128, 128], f32, tag="mB", name=f"mD_{c}")
                        mCv = mC[:].rearrange("p (b x) -> p b x", x=32)
                        mDv = mD[:].rearrange("p (b x) -> p b x", x=32)
                        nc.vector.tensor_mul(mCv, x2, cosv)
                        nc.vector.tensor_mul(mDv, x1, sinv)
                        nc.vector.tensor_add(rpv[:, :, 32:64], mCv, mDv)
                        nc.vector.tensor_copy(rpv[:, :, 64:128], pav[:, :, 64:128])
                        if apply_norm_w:
                            nc.vector.tensor_mul(rp[:], rp[:], normw_sb[:])

                        # scale by rstd, cast bf16 (DVE per-partition scalar)
                        qk_nat = workp.tile([128, 512], bf16, tag="qk_nat")
                        for b in range(4):
                            nc.vector.tensor_scalar_mul(
                                qk_nat[:, b * 128:(b + 1) * 128],
                                rp[:, b * 128:(b + 1) * 128], rstd[:, b:b + 1])

                        # k_scaled
                        for h in range(HPC):
                            nc.vector.tensor_scalar_mul(
                                ks_tb[:, cs + h * D:cs + (h + 1) * D],
                                qk_nat[:, 256 + h * 128:256 + (h + 1) * 128],
                                kdec_sb[:, h:h + 1])

                        # transpose q|k on the PE (SBUF->PSUM->SBUF, no DRAM trip)
                        psT = pstp.tile([128, 512], bf16, tag="psT")
                        for b in range(4):
                            nc.tensor.transpose(psT[:, b * 128:(b + 1) * 128],
                                                qk_nat[:, b * 128:(b + 1) * 128],
                                                ident_sb[:])
                        qkT = workp.tile([128, 512], bf16, tag="qkT", bufs=5,
                                         name=f"qkT_{c}")
                        nc.vector.tensor_copy(qkT[:], psT[:])
                        qk_tiles.append(qkT)

                    # ---------- phase 2: GLA scan ----------
                    for j in range(TB):
                        c = tb * TB + j
                        cs = j * HPC * D
                        sk_ps = pskp.tile([128, 512], f32, tag="sk")
                        st_ps = sk_ps[:, 0:256]
                        kv_ps = sk_ps[:, 256:512]
                        o_ps = pop.tile([128, HPC * D], f32, tag="o")
                        qkT = qk_tiles[j]
                        for h in range(HPC):
                            hh = h * 128
                            qTv = qkT[:, h * 128:(h + 1) * 128]
                            kTv = qkT[:, 256 + h * 128:256 + (h + 1) * 128]
                            vv = v_tb[:, cs + h * D:cs + (h + 1) * D]
                            ksv = ks_tb[:, cs + h * D:cs + (h + 1) * D]
                            # St[s,c] = k @ q^T
                            nc.tensor.matmul(st_ps[:, hh:hh + 128], kTv, qTv,
                                             start=True, stop=True)
                            At = workp.tile([128, 128], bf16, tag=f"At{h}",
                                            name=f"At{h}_{c}")
                            nc.vector.tensor_mul(At[:], st_ps[:, hh:hh + 128],
                                                 adect_sb[:, hh:hh + 128])
                            qs = workp.tile([128, 128], bf16, tag=f"qs{h}",
                                            name=f"qs{h}_{c}")
                            nc.vector.tensor_mul(qs[:], qTv, qdecb_sb[:, hh:hh + 128])
                            # o = At.T @ v + qs.T @ S
                            nc.tensor.matmul(o_ps[:, hh:hh + 128], At[:], vv,
                                             start=True, stop=False)
                            nc.tensor.matmul(o_ps[:, hh:hh + 128], qs[:], S_b[:, hh:hh + 128],
                                             start=False, stop=True)
                            # KV = ks.T @ v ; S = sdec*S + KV
                            nc.tensor.matmul(kv_ps[:, hh:hh + 128], ksv, vv,
                                             start=True, stop=True)
                            nc.vector.scalar_tensor_tensor(
                                S_sb[:, hh:hh + 128], S_sb[:, hh:hh + 128],
                                sdec_sb[:, h:h + 1], kv_ps[:, hh:hh + 128],
                                op0=ALU.mult, op1=ALU.add)
                            nc.vector.tensor_copy(S_b[:, hh:hh + 128], S_sb[:, hh:hh + 128])

                        # group rmsnorm (group == this core's 256 dims) + gate
                        gn_scr = workp.tile([128, HPC * D], bf16, tag="gn_scr")
                        gsumsq = workp.tile([128, 1], f32, tag="gsumsq")
                        nc.scalar.activation(gn_scr[:], o_ps[:], AF.Square,
                                             bias=zero_sb[:, 0:1], accum_out=gsumsq[:])
                        gsrt = workp.tile([128, 1], f32, tag="gsrt")
                        nc.scalar.activation(gsrt[:], gsumsq[:], AF.Sqrt,
                                             scale=1.0 / (HPC * D), bias=eps_sb[:, 0:1])
                        grstd = workp.tile([128, 1], f32, tag="grstd")
                        nc.vector.reciprocal(grstd[:], gsrt[:])
                        nc.vector.scalar_tensor_tensor(
                            attn_tb[:, cs:cs + HPC * D], o_ps[:], grstd[:],
                            gate_tb[:, cs:cs + HPC * D],
                            op0=ALU.mult, op1=ALU.mult)

                    # one batched attention write per t-block (3D dst AP)
                    c0_, c1_ = tb * TB, tb * TB + TB
                    seg = 0 if c1_ <= 32 else (1 if c1_ <= 48 else 2)
                    ch0 = c0_ - SEG_CH[seg][0]
                    seg_v3 = attn_seg[seg].ap().rearrange(
                        "(cc p) i -> p cc i", p=128)
                    attn_dma = nc.sync.dma_start(
                        seg_v3[:, ch0:ch0 + TB, :],
                        attn_tb[:].rearrange("p (cc i) -> p cc i", i=HPC * D))
                    if tb in (12, 14):
                        aT_anchor[0 if tb == 12 else 1] = attn_dma

                    # segment collectives fire as soon as their chunks are done
                    if tb == 7:
                        nc.gpsimd.collective_compute(
                            "AllToAll", ALU.bypass,
                            replica_groups=[list(range(N_CORES))],
                            ins=[attn_seg[0].ap().opt()],
                            outs=[att_x[0].ap().opt()],
                        )
                    if tb == 11:
                        nc.gpsimd.collective_compute(
                            "AllToAll", ALU.bypass,
                            replica_groups=[list(range(N_CORES))],
                            ins=[attn_seg[1].ap().opt()],
                            outs=[att_x[1].ap().opt()],
                        )
                    # dense lhsT prep issued mid-loop (after their collective
                    # is truly done, via anchors) so dense matmuls are ready
                    # the moment the phase loop drains
                    if tb == 13:
                        nat0 = emit_nat(0, anchor=aT_anchor[0])
                        aT0 = emit_aTpe(0, nat0)
                    if tb == 15:
                        nat1 = emit_nat(1, anchor=aT_anchor[1])
                        aT1 = emit_aTpe(1, nat1)

                # epilogue: final state out
                for h in range(HPC):
                    nc.sync.dma_start(state_out[h * D:(h + 1) * D, :],
                                      S_sb[:, h * D:(h + 1) * D])

            # last all-to-all (small: 2048 rows)
            nc.gpsimd.collective_compute(
                "AllToAll", ALU.bypass,
                replica_groups=[list(range(N_CORES))],
                ins=[attn_seg[2].ap().opt()],
                outs=[att_x[2].ap().opt()],
            )
            with tc.tile_pool(name="psDp", bufs=2, space="PSUM") as pdp:
                emit_dense(0, aT0, pdp)
                nat2 = emit_nat(2)
                aT2 = emit_aTpe(2, nat2)
                emit_dense(1, aT1, pdp)
                emit_dense(2, aT2, pdp)

    nc.compile()
    return nc


_NC_CACHE: dict = {}


def _get_nc(apply_norm_w: bool):
    key = apply_norm_w
    if key not in _NC_CACHE:
        _NC_CACHE[key] = _build_nc(apply_norm_w)
    return _NC_CACHE[key]


def _host_constants(positions):
    gam = _slopes()  # [H] float64
    c = np.arange(CHUNK, dtype=np.float64)
    dscale = D ** -0.5
    dt_ = c[None, :] - c[:, None]  # [s, c]
    adect = (np.where(dt_ >= 0, np.exp(gam[:, None, None] * dt_[None]), 0.0) * dscale)
    qdec = np.exp(gam[:, None] * (c + 1.0)[None, :]) * dscale        # [H, c]
    kdec = np.exp(gam[:, None] * (CHUNK - 1.0 - c)[None, :])         # [H, s]
    sdec = np.exp(gam * CHUNK)                                       # [H]
    inv = 1.0 / (THETA ** (np.arange(0, RD, 2, dtype=np.float64) / RD))
    ang = positions.astype(np.float64)[:, None] * inv[None, :]       # [T, 32]
    cos = np.cos(ang).astype(np.float32)
    sin = np.sin(ang).astype(np.float32)
    cos4 = np.tile(cos, (1, 4)).astype(np.float32)                   # [T, 128]
    sin4 = np.tile(sin, (1, 4)).astype(np.float32)
    cossin = np.ascontiguousarray(np.concatenate([cos4, sin4], axis=1))
    return (adect.astype(np.float32), qdec.astype(np.float32),
            kdec.astype(np.float32), sdec.astype(np.float32), cossin)


def kernel(positions, hidden_states, recurrent_state, w_qkv, w_g, w_dense,
           q_norm_w, k_norm_w, g_norm_w):
    positions = np.asarray(positions)
    hidden_states = np.asarray(hidden_states, dtype=np.float32)
    recurrent_state = np.asarray(recurrent_state, dtype=np.float32)
    w_qkv = np.asarray(w_qkv, dtype=np.float32)
    w_g = np.asarray(w_g, dtype=np.float32)
    w_dense = np.asarray(w_dense, dtype=np.float32)
    q_norm_w = np.asarray(q_norm_w, dtype=np.float32)
    k_norm_w = np.asarray(k_norm_w, dtype=np.float32)
    g_norm_w = np.asarray(g_norm_w, dtype=np.float32)

    apply_norm_w = not (np.all(q_norm_w == 1.0) and np.all(k_norm_w == 1.0))
    nc = _get_nc(apply_norm_w)

    adect, qdec, kdec, sdec, cossin = _host_constants(positions)
    hT_b = np.ascontiguousarray(hidden_states.T).astype(BF16_NP)
    w_dense_sc = (w_dense * g_norm_w[:, None]).astype(BF16_NP)

    in_maps = []
    for core in range(N_CORES):
        heads = [core * HPC + i for i in range(HPC)]
        cols = []
        for h in heads:
            cols.append(w_qkv[:, h * D:(h + 1) * D])
        for h in heads:
            cols.append(w_qkv[:, H * D + h * D:H * D + (h + 1) * D])
        for h in heads:
            cols.append(w_qkv[:, 2 * H * D + h * D:2 * H * D + (h + 1) * D])
        cols.append(w_g[:, core * HPC * D:(core + 1) * HPC * D])
        w_pack = np.ascontiguousarray(np.concatenate(cols, axis=1)).astype(BF16_NP)

        adect_c = np.ascontiguousarray(
            adect[heads].reshape(HPC * CHUNK, CHUNK))
        # qdecb: [HPC*D, CHUNK], broadcast of qdec over d
        qdecb_c = np.ascontiguousarray(
            np.broadcast_to(qdec[heads][:, None, :], (HPC, D, CHUNK))
            .reshape(HPC * D, CHUNK))
        kdec_c = np.ascontiguousarray(kdec[heads].T)                  # [CHUNK, HPC]
        sdec_c = np.ascontiguousarray(
            np.broadcast_to(sdec[heads][None, :], (CHUNK, HPC))).astype(np.float32)
        state_c = np.ascontiguousarray(
            recurrent_state[heads].reshape(HPC * D, D))

        m = {
            "hT_b": hT_b,
            "w_pack": w_pack,
            "w_dense_sc": w_dense_sc,
            "cossin": cossin,
            "adect": adect_c,
            "qdecb": qdecb_c,
            "kdec": kdec_c,
            "sdec": sdec_c,
            "state_in": state_c,
            "ident": np.eye(128, dtype=np.float32).astype(BF16_NP),
        }
        if apply_norm_w:
            nw = np.concatenate([q_norm_w, q_norm_w, k_norm_w, k_norm_w])
            m["normw"] = np.ascontiguousarray(
                np.broadcast_to(nw[None, :], (CHUNK, 512))).astype(np.float32)
        in_maps.append(m)

    global _last_in_maps
    _last_in_maps = in_maps
    try:
        res = bass_utils.run_bass_kernel_spmd(nc, in_maps,
                                              core_ids=list(range(N_CORES)))
    except Exception:
        # transient device errors (e.g. NRT_EXEC_UNIT_UNRECOVERABLE) usually
        # clear on retry
        import time as _time
        _time.sleep(10)
        res = bass_utils.run_bass_kernel_spmd(nc, in_maps,
                                              core_ids=list(range(N_CORES)))
    results = res.results

    # out_slice[c] = [t in [512c,512c+512) ; [4096+256c,+256) ; [6144+256c,+256)]
    out = np.empty((T, HID), dtype=np.float32)
    for c in range(N_CORES):
        sl = results[c]["out_slice"]
        out[512 * c:512 * (c + 1)] = sl[0:512]
        out[4096 + 256 * c:4096 + 256 * (c + 1)] = sl[512:768]
        out[6144 + 256 * c:6144 + 256 * (c + 1)] = sl[768:1024]
    new_state = np.concatenate(
        [results[c]["state_out"] for c in range(N_CORES)], axis=0
    ).reshape(H, D, D).astype(np.float32)
    return out, new_state
